# revision 1
# baseline (speedup 1.0000x reference)
"""ColorHistogramLoss Trainium2 kernel (8 NeuronCores, SPMD).

Sharding: 24 channels total (pred 12 + target 12, channel = (tensor,b,c));
core i owns channels {3i, 3i+1, 3i+2}, each laid out [128, 2048] f32.
A tiny AllGather shares per-channel (cumsum, 1/total) rows; every core
then computes the identical scalar loss on-device.

Two kernel variants (BASS_HIST_V env, default 2):

v2 (default) - Fourier factorization.  The soft histogram
    hist_j = sum_p exp(-(64 x_p - j - 0.5)^2 / 2)
is computed from 42 Fourier harmonics of the periodized Gaussian
(period 1.25 in x): S_t = sum_p e^{i 2 pi t 0.8 x_p} for t = 7*t1 + t2
factorizes as a rank-1-update matmul S = B^T A with per-pixel rows
B = e^{i phi 7 t1}, A = e^{i phi t2} (f16).  Per 128-pixel group one
LDWEIGHTS+MATMUL pair accumulates S in PSUM.  Trig args are range-reduced
with the DVE's round-to-nearest f32->int32 cast (frac in [-0.5, 0.5]) and
evaluated by ScalarE Sin (the int-cast is split DVE/ScalarE to balance
engine load); a small fixed matmul (synthesis matrix K) turns S into the
64-bin histograms.  Truncation+f16 error on the final loss is ~2e-5 rel.

v1 - dense: one ScalarE Derivative_Erf activation per (channel, bin) with
fused accum_out; correct but ScalarE-bound (~470 us vs ~290 us for v2).
"""

import math
from contextlib import ExitStack

import numpy as np

import concourse.bass as bass
import concourse.mybir as mybir
from concourse.bass_utils import run_bass_kernel_spmd

BINS = 64
N_CORES = 8
CH_PER_CORE = 3  # 24 channels / 8 cores
P = 128
G = 2048  # 512*512 / 128
PACK = BINS + 1  # per-channel payload: 64 cumsum values + 1 reciprocal
F32 = mybir.dt.float32

SCALE = 64.0 / math.sqrt(2.0)
INV_SQRT2 = 1.0 / math.sqrt(2.0)
# reference adds 1e-8 to the hist sum; our hist carries an extra 2/sqrt(pi)
EPS = (2.0 / math.sqrt(math.pi)) * 1e-8
AX = mybir.AxisListType
OP = mybir.AluOpType
AF = mybir.ActivationFunctionType


def build_nc():
    nc = bass.Bass(num_devices=N_CORES)

    x_ext = nc.declare_dram_parameter("x", [CH_PER_CORE, P, G], F32, isOutput=False)
    out_ext = nc.declare_dram_parameter("out", [1, 1], F32, isOutput=True)

    cdf_in = nc.dram_tensor("cdf_in", [1, CH_PER_CORE * PACK], F32)
    cdf_all = nc.dram_tensor(
        "cdf_all", [N_CORES, CH_PER_CORE * PACK], F32, addr_space="Shared"
    )

    bias_np = np.tile(
        (-(np.arange(BINS, dtype=np.float64) + 0.5) * INV_SQRT2).astype(np.float32),
        (P, 1),
    )
    bias_dram = nc.inline_tensor(bias_np, name="bias_const")

    with ExitStack() as stack:
        e = stack.enter_context
        xs = e(nc.sbuf_tensor("xs", [P, CH_PER_CORE * G], F32))
        scratch = e(nc.sbuf_tensor("scratch", [P, G], F32))
        acc = e(nc.sbuf_tensor("acc", [P, CH_PER_CORE * BINS], F32))
        ones_sb = e(nc.sbuf_tensor("ones", [P, 1], F32))
        biases_sb = e(nc.sbuf_tensor("biases", [P, BINS], F32))
        hrow = e(nc.sbuf_tensor("hrow", [1, CH_PER_CORE * BINS], F32))
        hsum = e(nc.sbuf_tensor("hsum", [1, CH_PER_CORE], F32))
        rinv = e(nc.sbuf_tensor("rinv", [1, CH_PER_CORE], F32))
        packed = e(nc.sbuf_tensor("packed", [1, CH_PER_CORE * PACK], F32))
        gp = e(nc.sbuf_tensor("gp", [12, PACK], F32))
        gt = e(nc.sbuf_tensor("gt", [12, PACK], F32))
        t1 = e(nc.sbuf_tensor("t1", [12, BINS], F32))
        t2 = e(nc.sbuf_tensor("t2", [12, BINS], F32))
        ra = e(nc.sbuf_tensor("ra", [12, 1], F32))
        loss_sb = e(nc.sbuf_tensor("loss", [1, 1], F32))
        ph = e(nc.psum_tensor("ph", [1, CH_PER_CORE * BINS], F32))
        pl = e(nc.psum_tensor("pl", [1, 1], F32))
        dma_sem = e(nc.semaphore("dma_sem"))
        act_sem = e(nc.semaphore("act_sem"))
        pe_sem = e(nc.semaphore("pe_sem"))
        dve_sem = e(nc.semaphore("dve_sem"))
        ones_sem = e(nc.semaphore("ones_sem"))
        cc_sem = e(nc.semaphore("cc_sem"))
        block = e(nc.Block())

        @block.sync
        def _(sync: bass.BassEngine):
            sync.dma_start(out=biases_sb[:, :], in_=bias_dram[:, :]).then_inc(
                dma_sem, 16
            )
            for c in range(CH_PER_CORE):
                sync.dma_start(
                    out=xs[:, c * G : (c + 1) * G], in_=x_ext[c, :, :]
                ).then_inc(dma_sem, 16)
            # packed (cumsums + reciprocals) -> collective input
            sync.wait_ge(dve_sem, 1)
            sync.dma_start(out=cdf_in[:, :], in_=packed[:, :]).then_inc(dma_sem, 16)
            # gathered payload -> SBUF; pred and target into separate tiles
            sync.wait_ge(cc_sem, 1)
            sync.dma_start(out=gp[:, :], in_=cdf_all[0:4, :]).then_inc(dma_sem, 16)
            sync.dma_start(out=gt[:, :], in_=cdf_all[4:8, :]).then_inc(dma_sem, 16)
            # final scalar -> output
            sync.wait_ge(act_sem, 2)
            sync.dma_start(out=out_ext[:, :], in_=loss_sb[:, :]).then_inc(dma_sem, 16)

        @block.scalar
        def _(scalar: bass.BassScalarEngine):
            scalar.wait_ge(dma_sem, 64)
            for c in range(CH_PER_CORE):
                xin = xs[:, c * G : (c + 1) * G]
                for j in range(BINS):
                    ins = scalar.activation(
                        scratch[:, :],
                        xin,
                        AF.Derivative_Erf,
                        bias=biases_sb[:, j : j + 1],
                        scale=SCALE,
                        accum_out=acc[:, c * BINS + j : c * BINS + j + 1],
                    )
            ins.then_inc(act_sem, 1)
            # final: loss = pl / 768
            scalar.wait_ge(pe_sem, 2)
            scalar.mul(loss_sb[:, :], pl[:, :], 1.0 / 768.0).then_inc(act_sem, 1)

        @block.vector
        def _(vector: bass.BassVectorEngine):
            vector.memset(ones_sb[:, :], 1.0).then_inc(ones_sem, 1)
            vector.wait_ge(pe_sem, 1)
            vector.tensor_copy(hrow[:, :], ph[:, :])
            # per-channel totals -> + eps -> reciprocal
            vector.tensor_reduce(
                hsum[:, :],
                hrow.ap().rearrange("p (c j) -> p c j", c=CH_PER_CORE),
                AX.X,
                OP.add,
            )
            vector.scalar_tensor_tensor(
                hsum[:, :], hsum[:, :], EPS, hsum[:, :], OP.add, OP.bypass
            )
            vector.reciprocal(rinv[:, :], hsum[:, :])
            # per-channel cumsum into the packed payload
            for c in range(CH_PER_CORE):
                vector.tensor_tensor_scan(
                    packed[:, c * PACK : c * PACK + BINS],
                    hrow[:, c * BINS : (c + 1) * BINS],
                    hrow[:, c * BINS : (c + 1) * BINS],
                    0.0,
                    OP.add,
                    OP.bypass,
                )
            # reciprocals into slot 64 of each channel payload
            vector.tensor_copy(
                packed[:, BINS :: PACK],
                rinv[:, :],
            ).then_inc(dve_sem, 1)
            # ----- loss stage (after the all-gather) -----
            vector.wait_ge(dma_sem, 112)
            vector.scalar_tensor_tensor(
                t1[:, :],
                gp[:, 0:BINS],
                gp[:, BINS : BINS + 1],
                gp[:, 0:BINS],
                OP.mult,
                OP.bypass,
            )
            vector.scalar_tensor_tensor(
                t2[:, :],
                gt[:, 0:BINS],
                gt[:, BINS : BINS + 1],
                t1[:, :],
                OP.mult,
                OP.subtract,
            )
            vector.tensor_reduce(
                ra[:, :], t2[:, :], AX.X, OP.add, apply_absolute_value=True
            ).then_inc(dve_sem, 1)

        @block.tensor
        def _(tensor: bass.BassTensorEngine):
            tensor.wait_ge(ones_sem, 1)
            tensor.wait_ge(act_sem, 1)
            tensor.matmul(
                ph[:, :], ones_sb[:, 0:1], acc[:, :], start=True, stop=True
            ).then_inc(pe_sem, 1)
            tensor.wait_ge(dve_sem, 2)
            tensor.matmul(
                pl[0:1, 0:1], ones_sb[0:12, 0:1], ra[0:12, 0:1], start=True, stop=True
            ).then_inc(pe_sem, 1)

        @block.gpsimd
        def _(gpsimd: bass.BassGpSimd):
            gpsimd.wait_ge(dma_sem, 80)
            gpsimd.collective_compute(
                "AllGather",
                OP.bypass,
                replica_groups=[list(range(N_CORES))],
                ins=[cdf_in.ap()],
                outs=[cdf_all.ap()],
            ).then_inc(cc_sem, 1)

    return nc


T1G = 6  # t1 grid (B side), harmonics at t = T2G*t1
T2G = 7  # t2 grid (A side)
NA = 2 * T2G  # A planes
NB = 2 * T1G  # B planes
BPAD = 16  # padded b-stride for the S-vector partition mapping


def _host_k_matrix():
    """Synthesis matrix Ksb [128, 129]: col 0 = Sin bias (0), cols 1+h*64+j = K.

    hist_j = sum_t Re[(2-d_t0) c_t e^{-i pi t/Pb} e^{-i 2 pi t j/Pb} S_t],
    t = T2G*t1 + t2;  S from the raw [2*T1G, 2*T2G] trig-product block,
    vectorized as v = a*BPAD + b (a = S row, b = S col), halves h of 128.
    """
    Pb = 64 * 1.25
    K = np.zeros((2 * T1G, BPAD, BINS), np.float64)
    j = np.arange(BINS)
    for t1 in range(T1G):
        for t2 in range(T2G):
            t = t1 * T2G + t2
            ct = (math.sqrt(2 * math.pi) / Pb) * math.exp(
                -0.5 * (2 * math.pi * t / Pb) ** 2
            )
            mult = 1.0 if t == 0 else 2.0
            w = mult * ct * np.exp(-1j * 2 * np.pi * t * (0.5 + j) / Pb)
            K[2 * t1, 2 * t2] = w.real
            K[2 * t1 + 1, 2 * t2 + 1] = -w.real
            K[2 * t1, 2 * t2 + 1] = -w.imag
            K[2 * t1 + 1, 2 * t2, :] = -w.imag
    # sin planes stored HALVED for squared harmonics t1=4 (a=9), t2=4 (b=9),
    # t2=6 (b=13); compensate here.
    K[9, :, :] *= 2.0
    K[:, 9, :] *= 2.0
    K[:, 13, :] *= 2.0
    consts = np.zeros((128, 129), np.float32)
    for p in range(128):
        for h in range(2):
            a = h * 8 + p // BPAD
            b = p % BPAD
            if a < 2 * T1G:
                consts[p, 1 + h * 64 : 1 + (h + 1) * 64] = K[a, b]
    return consts


GS = 1024  # max groups (of 128 pixels) per strip; plane stride
# graduated first strips so the PE can start early
STRIPS = [(0, 0, 256), (0, 256, 256), (0, 512, 512), (0, 1024, 1024)] + [
    (1, 0, 1024), (1, 1024, 1024),
    (2, 0, 1024), (2, 1024, 512), (2, 1536, 512),
]
NSTRIPS = len(STRIPS)
NPAIR = 6  # harmonic pairs needing trig per strip (rest via DVE squaring)
F16 = mybir.dt.float16


def build_nc_fourier():
    nc = bass.Bass(num_devices=N_CORES)

    x_ext = nc.declare_dram_parameter("x", [CH_PER_CORE, P, G], F32, isOutput=False)
    out_ext = nc.declare_dram_parameter("out", [1, 1], F32, isOutput=True)

    cdf_in = nc.dram_tensor("cdf_in", [1, CH_PER_CORE * PACK], F32)
    cdf_all = nc.dram_tensor(
        "cdf_all", [N_CORES, CH_PER_CORE * PACK], F32, addr_space="Shared"
    )
    sdram = nc.dram_tensor("sdram", [2 * T1G, CH_PER_CORE, BPAD], F32)
    consts_dram = nc.inline_tensor(_host_k_matrix(), name="consts_k")

    # trig pair schedule: (t, A-or-B, plane-base); remaining harmonics are
    # derived on VectorE by complex squaring of these planes.
    pairs = [
        (1, "A", 2),
        (T2G, "B", 2),
        (3, "A", 6),
        (3 * T2G, "B", 6),
        (5, "A", 10),
        (5 * T2G, "B", 10),
    ]
    # squares: (side, src_plane, dst_plane, halve_sin, src_trig_pair_idx)
    squares = [
        ("A", 2, 4, False, 0),
        ("B", 2, 4, False, 1),
        ("A", 6, 12, True, 2),
        ("A", 4, 8, True, None),
        ("B", 4, 8, True, None),
    ]

    with ExitStack() as stack:
        e = stack.enter_context
        xs = e(nc.sbuf_tensor("xs", [P, CH_PER_CORE * G], F32))
        consts = e(nc.sbuf_tensor("consts", [P, 129], F32))
        # trig planes, double buffered: [A(16 planes) | B(16 planes)] * GS
        plA = [e(nc.sbuf_tensor(f"plA{b}", [P, NA * GS], F16)) for b in range(2)]
        plB = [e(nc.sbuf_tensor(f"plB{b}", [P, NB * GS], F16)) for b in range(2)]
        args = e(nc.sbuf_tensor("args", [P, 8 * GS], F32))  # fr/frc ring, 4 pairs
        irnd = e(nc.sbuf_tensor("irnd", [P, 2 * GS], mybir.dt.int32))
        i1r = e(nc.sbuf_tensor("i1r", [P, 4 * GS], mybir.dt.int32))  # ACT-cast ring
        sqsc = e(nc.sbuf_tensor("sqsc", [P, 2 * GS], F16))  # (a^2, b^2) scratch
        ones_sb = e(nc.sbuf_tensor("ones", [P, 1], F32))
        S_sb = e(nc.sbuf_tensor("S_sb", [2 * T1G, CH_PER_CORE * BPAD], F32))
        svec = e(nc.sbuf_tensor("svec", [P, 2 * CH_PER_CORE], F32))
        hsum = e(nc.sbuf_tensor("hsum", [CH_PER_CORE, 1], F32))
        rinv = e(nc.sbuf_tensor("rinv", [CH_PER_CORE, 1], F32))
        packed = e(nc.sbuf_tensor("packed", [CH_PER_CORE, PACK], F32))
        gp = e(nc.sbuf_tensor("gp", [12, PACK], F32))
        gt = e(nc.sbuf_tensor("gt", [12, PACK], F32))
        t1s = e(nc.sbuf_tensor("t1s", [12, BINS], F32))
        t2s = e(nc.sbuf_tensor("t2s", [12, BINS], F32))
        ra = e(nc.sbuf_tensor("ra", [12, 1], F32))
        loss_sb = e(nc.sbuf_tensor("loss", [1, 1], F32))
        ps = e(nc.psum_tensor("ps", [2 * T1G, CH_PER_CORE * 2 * T2G], F32))
        ph2 = e(nc.psum_tensor("ph2", [CH_PER_CORE, BINS], F32))
        pl = e(nc.psum_tensor("pl", [1, 1], F32))
        dma_sem = e(nc.semaphore("dma_sem"))
        dve_pair = e(nc.semaphore("dve_pair"))
        act_pair = e(nc.semaphore("act_pair"))
        act_i1 = e(nc.semaphore("act_i1"))
        pe_strip = e(nc.semaphore("pe_strip"))
        dve_sq = e(nc.semaphore("dve_sq"))
        pe_sem = e(nc.semaphore("pe_sem"))
        dve_sem = e(nc.semaphore("dve_sem"))
        act_sem = e(nc.semaphore("act_sem"))
        cc_sem = e(nc.semaphore("cc_sem"))
        block = e(nc.Block())

        def strip_x(s):
            c, o, sz = STRIPS[s]
            return xs[:, c * G + o : c * G + o + sz]

        def pview(pl_, NP, q, sz):
            # plane q as [128, sz//64, 64]: col = ghi*(NP*64) + q*64 + glo
            v = pl_.ap().rearrange("p (ghi q glo) -> p q ghi glo", q=NP, glo=64)
            return v[:, q, 0 : sz // 64, :]

        def blk(ap2d):
            return ap2d.rearrange("p (a b) -> p a b", b=64)

        @block.sync
        def _(sync: bass.BassEngine):
            sync.dma_start(out=consts[:, :], in_=consts_dram[:, :]).then_inc(
                dma_sem, 16
            )
            for c in range(CH_PER_CORE):
                sync.dma_start(
                    out=xs[:, c * G : (c + 1) * G], in_=x_ext[c, :, :]
                ).then_inc(dma_sem, 16)
            # S block -> DRAM -> svec (partition reshape)
            sync.wait_ge(dve_sem, 1)
            sync.dma_start(out=sdram[:, :, :], in_=S_sb[:, :]).then_inc(dma_sem, 16)
            for c in range(CH_PER_CORE):
                for h in range(2):
                    lo = h * 8
                    hi = min(2 * T1G, lo + 8)
                    sync.dma_start(
                        out=svec[0 : (hi - lo) * BPAD, 2 * c + h : 2 * c + h + 1],
                        in_=sdram[lo:hi, c, :],
                    ).then_inc(dma_sem, 16)
            # packed cdfs -> collective input
            sync.wait_ge(dve_sem, 2)
            sync.dma_start(out=cdf_in[:, :], in_=packed[:, :]).then_inc(dma_sem, 16)
            # gathered payload
            sync.wait_ge(cc_sem, 1)
            sync.dma_start(out=gp[:, :], in_=cdf_all[0:4, :]).then_inc(dma_sem, 16)
            sync.dma_start(out=gt[:, :], in_=cdf_all[4:8, :]).then_inc(dma_sem, 16)
            # final scalar
            sync.wait_ge(act_sem, 1)
            sync.dma_start(out=out_ext[:, :], in_=loss_sb[:, :]).then_inc(dma_sem, 16)

        @block.vector
        def _(vector: bass.BassVectorEngine):
            vector.memset(ones_sb[:, :], 1.0)
            vector.memset(svec[:, :], 0.0)
            vector.memset(S_sb[:, :], 0.0)
            # constant t=0 planes: cos=1, sin=0
            for b in range(2):
                vector.memset(pview(plA[b], NA, 0, GS), 1.0)
                vector.memset(pview(plA[b], NA, 1, GS), 0.0)
                vector.memset(pview(plB[b], NB, 0, GS), 1.0)
                vector.memset(pview(plB[b], NB, 1, GS), 0.0)
            vector.wait_ge(dma_sem, 64)
            for s in range(NSTRIPS):
                xin = strip_x(s)
                b = s % 2
                sz = STRIPS[s][2]

                def do_square(job, inc=False):
                    side, sq_, dq, halve, wk = job
                    pl_ = plA[b] if side == "A" else plB[b]
                    NP = NA if side == "A" else NB
                    if wk is not None:
                        vector.wait_ge(act_pair, s * NPAIR + wk + 1)
                    sc = pview(pl_, NP, sq_, sz)
                    sn = pview(pl_, NP, sq_ + 1, sz)
                    dc = pview(pl_, NP, dq, sz)
                    ds = pview(pl_, NP, dq + 1, sz)
                    # (cos|sin) planes are adjacent 128-wide pairs: square both
                    # in one op
                    pair = pl_.ap().rearrange(
                        "p (ghi qq glo) -> p qq ghi glo", qq=NP // 2, glo=128
                    )[:, sq_ // 2, 0 : sz // 64, :]
                    sq2 = sqsc.ap().rearrange("p (a b) -> p a b", b=128)[
                        :, 0 : sz // 64, :
                    ]
                    vector.tensor_mul(sq2, pair, pair)
                    vector.tensor_sub(dc, sq2[:, :, 0:64], sq2[:, :, 64:128])
                    ins = vector.tensor_mul(ds, sc, sn)
                    if not halve:
                        ins = vector.tensor_scalar(ds, ds, 2.0, None, OP.mult)
                    if inc:
                        ins.then_inc(dve_sq, 1)

                for k, (t, _, _) in enumerate(pairs):
                    kg = s * NPAIR + k
                    if kg >= 4:
                        vector.wait_ge(act_pair, kg - 3)
                    vector.wait_ge(act_i1, kg + 1)
                    slot = (kg % 4) * 2
                    fr = args[:, slot * GS : slot * GS + sz]
                    frc = args[:, (slot + 1) * GS : (slot + 1) * GS + sz]
                    ir = irnd[:, (kg % 2) * GS : (kg % 2) * GS + sz]
                    i1 = i1r[:, (kg % 4) * GS : (kg % 4) * GS + sz]
                    # fr = t*0.8*x - round(t*0.8*x)  in [-0.5, 0.5]
                    vector.scalar_tensor_tensor(
                        fr, xin, t * 0.8, i1, OP.mult, OP.subtract
                    )
                    # frc = (fr+0.25) - round(fr+0.25): phase shifted by +pi/2
                    vector.tensor_scalar(ir, fr, 0.25, None, OP.add)
                    vector.scalar_tensor_tensor(
                        frc, fr, 0.25, ir, OP.add, OP.subtract
                    ).then_inc(dve_pair, 1)
                    # interleave derived-harmonic squares once sources are up
                    if k == 2:
                        if s >= 2:
                            vector.wait_ge(pe_strip, s - 1)
                        do_square(squares[0])
                        do_square(squares[1])
                        do_square(squares[3])
                        do_square(squares[4])
                    elif k == 4:
                        do_square(squares[2], inc=True)
            # ---- after PE S-accumulation: normalize + cumsum + pack ----
            vector.wait_ge(pe_strip, NSTRIPS)
            for c in range(CH_PER_CORE):
                ins = vector.tensor_copy(
                    S_sb[:, c * BPAD : c * BPAD + 2 * T2G],
                    ps[:, c * 2 * T2G : (c + 1) * 2 * T2G],
                )
            ins.then_inc(dve_sem, 1)
            vector.wait_ge(pe_sem, 1)
            vector.tensor_reduce(hsum[:, :], ph2[:, :], AX.X, OP.add)
            vector.scalar_tensor_tensor(
                hsum[:, :], hsum[:, :], 1e-8, hsum[:, :], OP.add, OP.bypass
            )
            vector.reciprocal(rinv[:, :], hsum[:, :])
            vector.tensor_tensor_scan(
                packed[:, 0:BINS],
                ph2[:, :],
                t1s[0:CH_PER_CORE, :],
                0.0,
                OP.add,
                OP.bypass,
            )
            vector.tensor_copy(packed[:, BINS : BINS + 1], rinv[:, :]).then_inc(
                dve_sem, 1
            )
            # ---- loss stage ----
            vector.wait_ge(dma_sem, 224)
            vector.scalar_tensor_tensor(
                t1s[:, :],
                gp[:, 0:BINS],
                gp[:, BINS : BINS + 1],
                gp[:, 0:BINS],
                OP.mult,
                OP.bypass,
            )
            vector.scalar_tensor_tensor(
                t2s[:, :],
                gt[:, 0:BINS],
                gt[:, BINS : BINS + 1],
                t1s[:, :],
                OP.mult,
                OP.subtract,
            )
            vector.tensor_reduce(
                ra[:, :], t2s[:, :], AX.X, OP.add, apply_absolute_value=True
            ).then_inc(dve_sem, 1)

        @block.scalar
        def _(scalar: bass.BassScalarEngine):
            scalar.wait_ge(dma_sem, 64)
            TWO_PI = 2.0 * math.pi
            NKG = NSTRIPS * NPAIR
            all_pairs = [(s, k) + pairs[k] for s in range(NSTRIPS) for k in range(NPAIR)]

            def emit_i1(kg):
                s, k, t, _, _ = all_pairs[kg]
                sz = STRIPS[s][2]
                i1 = i1r[:, (kg % 4) * GS : (kg % 4) * GS + sz]
                return scalar.activation(
                    i1, strip_x(s), AF.Copy, bias=0.0, scale=t * 0.8
                ).then_inc(act_i1, 1)

            emit_i1(0)
            emit_i1(1)
            for kg in range(NKG):
                s, k, t, which, q = all_pairs[kg]
                b = s % 2
                if kg + 2 < NKG:
                    if kg >= 2:
                        scalar.wait_ge(dve_pair, kg - 1)
                    emit_i1(kg + 2)
                if k == 0 and s >= 2:
                    scalar.wait_ge(pe_strip, s - 1)
                scalar.wait_ge(dve_pair, kg + 1)
                sz = STRIPS[s][2]
                slot = (kg % 4) * 2
                fr = args[:, slot * GS : slot * GS + sz]
                frc = args[:, (slot + 1) * GS : (slot + 1) * GS + sz]
                pl_ = plA[b] if which == "A" else plB[b]
                NP = NA if which == "A" else NB
                # cos into plane q, sin into plane q+1
                scalar.activation(
                    pview(pl_, NP, q, sz),
                    blk(frc),
                    AF.Sin,
                    bias=consts[:, 0:1],
                    scale=TWO_PI,
                )
                scalar.activation(
                    pview(pl_, NP, q + 1, sz),
                    blk(fr),
                    AF.Sin,
                    bias=consts[:, 0:1],
                    scale=TWO_PI,
                ).then_inc(act_pair, 1)
            # final: loss = pl / 768
            scalar.wait_ge(pe_sem, 2)
            scalar.mul(loss_sb[:, :], pl[:, :], 1.0 / 768.0).then_inc(act_sem, 1)

        @block.tensor
        def _(tensor: bass.BassTensorEngine):
            for s in range(NSTRIPS):
                b = s % 2
                c, o, sz = STRIPS[s]
                tensor.wait_ge(act_pair, NPAIR * (s + 1))
                tensor.wait_ge(dve_sq, s + 1)
                for g in range(sz):
                    ghi, glo = divmod(g, 64)
                    bb = ghi * (NB * 64) + glo
                    aa = ghi * (NA * 64) + glo
                    ins = tensor.matmul(
                        ps[:, 2 * T2G * c : 2 * T2G * (c + 1)],
                        plB[b][:, bb : bb + (NB - 1) * 64 + 1 : 64],
                        plA[b][:, aa : aa + (NA - 1) * 64 + 1 : 64],
                        start=(o == 0 and g == 0),
                        stop=(o + sz == G and g == sz - 1),
                    )
                ins.then_inc(pe_strip, 1)
            # synthesis: ph2[c, j] = sum_h sum_p svec[p, 2c+h] * K[p, h*64+j]
            tensor.wait_ge(dma_sem, 176)
            for h in range(2):
                ins = tensor.matmul(
                    ph2[:, :],
                    svec[:, h :: 2],
                    consts[:, 1 + h * BINS : 1 + (h + 1) * BINS],
                    start=(h == 0),
                    stop=(h == 1),
                )
            ins.then_inc(pe_sem, 1)
            tensor.wait_ge(dve_sem, 3)
            tensor.matmul(
                pl[0:1, 0:1], ones_sb[0:12, 0:1], ra[0:12, 0:1], start=True, stop=True
            ).then_inc(pe_sem, 1)  # pe_sem reaches 2

        @block.gpsimd
        def _(gpsimd: bass.BassGpSimd):
            gpsimd.wait_ge(dma_sem, 192)
            gpsimd.collective_compute(
                "AllGather",
                OP.bypass,
                replica_groups=[list(range(N_CORES))],
                ins=[cdf_in.ap()],
                outs=[cdf_all.ap()],
            ).then_inc(cc_sem, 1)

    return nc


import os

_VERSION = int(os.environ.get("BASS_HIST_V", "2"))
_NC_CACHE = None


def _get_nc():
    global _NC_CACHE
    if _NC_CACHE is None:
        _NC_CACHE = build_nc_fourier() if _VERSION == 2 else build_nc()
    return _NC_CACHE


def kernel(pred: np.ndarray, target: np.ndarray) -> np.ndarray:
    assert pred.shape == (4, 3, 512, 512) and target.shape == (4, 3, 512, 512)
    chans = np.concatenate(
        [
            np.ascontiguousarray(pred, dtype=np.float32).reshape(12, P, G),
            np.ascontiguousarray(target, dtype=np.float32).reshape(12, P, G),
        ],
        axis=0,
    )  # [24, 128, 2048]

    in_maps = [
        {"x": np.ascontiguousarray(chans[3 * i : 3 * i + 3])} for i in range(N_CORES)
    ]

    nc = _get_nc()
    last_err = None
    for _attempt in range(3):
        try:
            res = run_bass_kernel_spmd(nc, in_maps, core_ids=list(range(N_CORES)))
            out = res.results[0]["out"]
            return np.asarray(out, dtype=np.float32).reshape(())
        except Exception as err:  # transient NRT_EXEC_UNIT_UNRECOVERABLE retries
            last_err = err
    raise last_err



# revision 33
# speedup vs baseline: 2.0203x; 2.0203x over previous
"""ColorHistogramLoss Trainium2 kernel (8 NeuronCores, SPMD).

Sharding: 24 channels total (pred 12 + target 12, channel = (tensor,b,c));
core i owns channels {3i, 3i+1, 3i+2}, each laid out [128, 2048] f32.

v3 (default) - 12-harmonic Fourier factorization, period Pb=70 bins.
The soft histogram hist_j = sum_p exp(-(64 x_p - j - 0.5)^2 / 2) is
approximated from harmonics t = 3*t1 + t2 (t1<4, t2<3) of the periodized
Gaussian: S(t1,t2) = sum_p B_t1(x_p) A_t2(x_p), a bilinear form computed by
the PE in block-diagonal supergroups of M=16 pixel-groups per LDW+MATMUL
(out [8*16, 6*16], useful diagonal blocks accumulated in PSUM; 128
instruction pairs per channel instead of 2048).

Range reduction needs no int casts: the DVE tensor_scalar two-scalar form
fr0 = (x * t*64/70) mod 1 does it in one op; sin/cos come from ScalarE
Sin with args kept in [-pi, pi] via sign absorption into the synthesis
matrix K2 (stored planes are -sin / -cos). A2/B6 planes derive from
A1/B3 by f16 squaring (sin stored halved, compensated in K2); B9 is
direct trig from 3*fr0(B3) mod 1. All planes f16.

Each core AllGathers its raw S block (8x18 f32), then every core runs the
identical tail: one synthesis matmul for all 24 channels, normalize,
cumsum, loss. Simulated accuracy of this exact scheme: rel err ~3e-4.

v2 - previous 42-harmonic version (BASS_HIST_V=2), ~290-360us measured.
"""

import math
import os
from contextlib import ExitStack

import numpy as np

import concourse.bass as bass
import concourse.mybir as mybir
from concourse.bass_utils import run_bass_kernel_spmd

BINS = 64
N_CORES = 8
CH_PER_CORE = 3
P = 128
G = 2048  # 512*512 / 128
F32 = mybir.dt.float32
F16 = mybir.dt.float16
AX = mybir.AxisListType
OP = mybir.AluOpType
AF = mybir.ActivationFunctionType

# ---------------- v3: 12-harmonic scheme ----------------
PB = 70.0
SCALE = 64.0 / PB          # phase per unit x for t=1
NA = 6                     # A planes: (c0,s0,cA1,sA1,cA2,sA2h)
NB = 8                     # B planes: (c0,s0,cB3,sB3,cB6,sB6h,cB9,sB9)
MSG = 16                   # supergroup: groups per LDWEIGHTS+MATMUL
GS3 = 512                  # plane buffer capacity (groups)
TWO_PI = 2.0 * math.pi
# per-channel strips (offset, size); sizes multiples of 64
CH_STRIPS = [(0, 256), (256, 256), (512, 512), (1024, 512), (1536, 512)]
STRIPS3 = [(c, o, sz) for c in range(CH_PER_CORE) for (o, sz) in CH_STRIPS]
NSTRIPS3 = len(STRIPS3)
# dma_sem threshold for each strip's x data (ch0a=16, ch0b=32, ch1=48, ch2=64)
def _strip_dma_need(c, o):
    if c == 0:
        return 16 if o == 0 else 32
    return 48 if c == 1 else 64


def _host_k2():
    """Synthesis matrix K2 [48, 64]: row (qb*6+qa), col j.

    hist_j = sum_{t1,t2} Re[w(t) * S(t1,t2)] with S from the stored planes;
    trig planes hold true cos/sin, derived planes (A2, B6) store sin halved.
    """
    jj = np.arange(BINS)
    # (sign_c, sign_s): stored_c = sign_c * cos, stored_s = sign_s * sin
    sgA = [(1.0, 1.0), (1.0, 1.0), (1.0, 0.5)]
    sgB = [(1.0, 1.0), (1.0, 1.0), (1.0, 0.5), (1.0, 1.0)]
    K = np.zeros((NB, NA, BINS))
    for t1 in range(4):
        for t2 in range(3):
            t = 3 * t1 + t2
            ct = (math.sqrt(2 * math.pi) / PB) * math.exp(
                -0.5 * (2 * math.pi * t / PB) ** 2
            )
            mult = 1.0 if t == 0 else 2.0
            w = mult * ct * np.exp(-1j * 2 * np.pi * t * (0.5 + jj) / PB)
            K[2 * t1, 2 * t2] += w.real / (sgB[t1][0] * sgA[t2][0])
            K[2 * t1 + 1, 2 * t2 + 1] += -w.real / (sgB[t1][1] * sgA[t2][1])
            K[2 * t1 + 1, 2 * t2] += -w.imag / (sgB[t1][1] * sgA[t2][0])
            K[2 * t1, 2 * t2 + 1] += -w.imag / (sgB[t1][0] * sgA[t2][1])
    return np.ascontiguousarray(K.reshape(NB * NA, BINS).astype(np.float32))


def _host_mask():
    """Diagonal mask [128, 288]: 1 where psum col's m' == partition's m."""
    m = np.zeros((P, CH_PER_CORE * NA * MSG), np.float32)
    for p in range(P):
        for col in range(CH_PER_CORE * NA * MSG):
            if col % MSG == p % MSG:
                m[p, col] = 1.0
    return m


def _host_sel():
    """Block selector [128, 8]: SEL[p, qb] = 1 iff p // 16 == qb."""
    s = np.zeros((P, NB), np.float32)
    for p in range(P):
        s[p, p // MSG] = 1.0
    return s


def build_nc_v3():
    nc = bass.Bass(num_devices=N_CORES)

    x_ext = nc.declare_dram_parameter("x", [CH_PER_CORE, P, G], F32, isOutput=False)
    out_ext = nc.declare_dram_parameter("out", [1, 1], F32, isOutput=True)

    cdf_in = nc.dram_tensor("cdf_in", [NB, CH_PER_CORE * NA], F32)
    cdf_all = nc.dram_tensor(
        "cdf_all", [N_CORES * NB, CH_PER_CORE * NA], F32, addr_space="Shared"
    )
    k2_dram = nc.inline_tensor(_host_k2(), name="k2_const")
    mask_dram = nc.inline_tensor(_host_mask(), name="mask_const")
    sel_dram = nc.inline_tensor(_host_sel(), name="sel_const")

    with ExitStack() as stack:
        e = stack.enter_context
        xs = e(nc.sbuf_tensor("xs", [P, CH_PER_CORE * G], F32))
        xh = e(nc.sbuf_tensor("xh", [P, CH_PER_CORE * G], F16))
        # 16-interleaved layout: col = blk*(NP*16) + q*16 + (g%16)
        plA = [e(nc.sbuf_tensor(f"plA{b}", [P, GS3 * NA], F16)) for b in range(2)]
        plB = [e(nc.sbuf_tensor(f"plB{b}", [P, GS3 * NB], F16)) for b in range(2)]
        # args ring: 6 slots/strip (t, t2 for the 3 trig pairs), 2 bufs
        args = e(nc.sbuf_tensor("args", [P, 12 * GS3], F16))
        usc = e(nc.sbuf_tensor("usc", [P, GS3], F16))
        rsc = e(nc.sbuf_tensor("rsc", [P, GS3], F16))
        sqt = e(nc.sbuf_tensor("sqt", [P, GS3], F16))
        P3sb = e(nc.sbuf_tensor("P3sb", [P, CH_PER_CORE * NA * MSG], F32))
        R2sb = e(nc.sbuf_tensor("R2sb", [P, CH_PER_CORE * NA], F32))
        masksb = e(nc.sbuf_tensor("masksb", [P, CH_PER_CORE * NA * MSG], F32))
        selsb = e(nc.sbuf_tensor("selsb", [P, NB], F32))
        S3 = e(nc.sbuf_tensor("S3", [NB, CH_PER_CORE * NA], F32))
        SV = e(nc.sbuf_tensor("SV", [NB * NA, N_CORES * CH_PER_CORE], F32))
        k2sb = e(nc.sbuf_tensor("k2sb", [NB * NA, BINS], F32))
        cdfs = e(nc.sbuf_tensor("cdfs", [12, 2 * BINS], F32))
        hsum = e(nc.sbuf_tensor("hsum", [12, 2], F32))
        rinv = e(nc.sbuf_tensor("rinv", [12, 2], F32))
        t1s = e(nc.sbuf_tensor("t1s", [12, BINS], F32))
        ra = e(nc.sbuf_tensor("ra", [12, 1], F32))
        ones_sb = e(nc.sbuf_tensor("ones", [P, 1], F32))
        pospi2 = e(nc.sbuf_tensor("pospi2", [P, 1], F32))
        zerob = e(nc.sbuf_tensor("zerob", [P, 1], F32))
        loss_sb = e(nc.sbuf_tensor("loss", [1, 1], F32))
        ps = e(nc.psum_tensor("ps", [P, CH_PER_CORE * NA * MSG], F32))
        S2 = e(nc.psum_tensor("S2", [NB, CH_PER_CORE * NA], F32))
        ph2 = e(nc.psum_tensor("ph2", [12, 2 * BINS], F32))
        pl = e(nc.psum_tensor("pl", [1, 1], F32))
        dma_sem = e(nc.semaphore("dma_sem"))
        act_xh = e(nc.semaphore("act_xh"))
        dve_pair = e(nc.semaphore("dve_pair"))
        act_pair = e(nc.semaphore("act_pair"))
        dve_sq = e(nc.semaphore("dve_sq"))
        pe_strip = e(nc.semaphore("pe_strip"))
        dve_sem = e(nc.semaphore("dve_sem"))
        pe_sem = e(nc.semaphore("pe_sem"))
        act_sem = e(nc.semaphore("act_sem"))
        cc_sem = e(nc.semaphore("cc_sem"))
        block = e(nc.Block())

        def xv(c, o, sz):
            return xs[:, c * G + o : c * G + o + sz]

        def xhv(c, o, sz):
            return xh[:, c * G + o : c * G + o + sz]

        def pview(pl_, NP, q, sz):
            # plane q over the strip: [p, sz/16, 16] (16-contiguous runs)
            v = pl_.ap().rearrange("p (blk q g) -> p q blk g", q=NP, g=16)
            return v[:, q, 0 : sz // 16, :]

        def slot(s, k, sz):
            base = ((s % 2) * 6 + k) * GS3
            return args[:, base : base + sz]

        @block.sync
        def _(sync: bass.BassEngine):
            sync.dma_start(out=xs[:, 0:256], in_=x_ext[0, :, 0:256]).then_inc(
                dma_sem, 16
            )
            sync.dma_start(out=xs[:, 256:G], in_=x_ext[0, :, 256:G]).then_inc(
                dma_sem, 16
            )
            sync.dma_start(out=xs[:, G : 2 * G], in_=x_ext[1, :, :]).then_inc(
                dma_sem, 16
            )
            sync.dma_start(out=xs[:, 2 * G : 3 * G], in_=x_ext[2, :, :]).then_inc(
                dma_sem, 16
            )
            sync.dma_start(out=k2sb[:, :], in_=k2_dram[:, :]).then_inc(dma_sem, 16)
            sync.dma_start(out=masksb[:, :], in_=mask_dram[:, :]).then_inc(
                dma_sem, 16
            )
            sync.dma_start(out=selsb[:, :], in_=sel_dram[:, :]).then_inc(dma_sem, 16)
            # raw S block -> collective input
            sync.wait_ge(dve_sem, 2)
            sync.dma_start(out=cdf_in[:, :], in_=S3[0:NB, :]).then_inc(dma_sem, 16)
            # gathered S -> SV [48, 24]: SV[qb*6+qa, core*3+c] = cdf_all[core*8+qb, qa*3+c]
            sync.wait_ge(cc_sem, 1)
            gat = cdf_all.ap().rearrange(
                "(core qb) (qa c) -> qb qa core c", qb=NB, c=CH_PER_CORE
            )
            for qb in range(NB):
                sync.dma_start(
                    out=SV[qb * NA : (qb + 1) * NA, :], in_=gat[qb]
                ).then_inc(dma_sem, 16)
            # final scalar
            sync.wait_ge(act_sem, 1)
            sync.dma_start(out=out_ext[:, :], in_=loss_sb[:, :]).then_inc(dma_sem, 16)

        @block.scalar
        def _(scalar: bass.BassScalarEngine):
            def emit_xh(s):
                c, o, sz = STRIPS3[s]
                scalar.wait_ge(dma_sem, _strip_dma_need(c, o))
                scalar.activation(xhv(c, o, sz), xv(c, o, sz), AF.Copy).then_inc(
                    act_xh, 1
                )

            emit_xh(0)
            for s in range(NSTRIPS3):
                c, o, sz = STRIPS3[s]
                b = s % 2
                if s + 1 < NSTRIPS3:
                    emit_xh(s + 1)
                if s >= 2:
                    scalar.wait_ge(pe_strip, s - 1)  # plane buffer reuse
                # trig pair k: cos -> plane q (from t2), sin -> q+1 (from t)
                for k, (pl_, NP, q) in enumerate(
                    [(plA[b], NA, 2), (plB[b], NB, 2), (plB[b], NB, 6)]
                ):
                    scalar.wait_ge(dve_pair, 3 * s + k + 1)
                    t = slot(s, 2 * k, sz)
                    t2 = slot(s, 2 * k + 1, sz)
                    tv = t.rearrange("p (blk g) -> p blk g", g=16)
                    t2v = t2.rearrange("p (blk g) -> p blk g", g=16)
                    scalar.activation(
                        pview(pl_, NP, q, sz), t2v, AF.Sin,
                        bias=pospi2[:, 0:1], scale=TWO_PI,
                    )
                    scalar.activation(
                        pview(pl_, NP, q + 1, sz), tv, AF.Sin,
                        bias=zerob[:, 0:1], scale=TWO_PI,
                    ).then_inc(act_pair, 1)
            # final: loss = pl / 768
            scalar.wait_ge(pe_sem, 3)
            scalar.mul(loss_sb[:, :], pl[:, :], 1.0 / 768.0).then_inc(act_sem, 1)

        @block.vector
        def _(vector: bass.BassVectorEngine):
            vector.memset(ones_sb[:, :], 1.0)
            vector.memset(pospi2[:, :], math.pi / 2.0)
            vector.memset(zerob[:, :], 0.0)
            for b in range(2):
                vector.memset(pview(plA[b], NA, 0, GS3), 1.0)
                vector.memset(pview(plA[b], NA, 1, GS3), 0.0)
                vector.memset(pview(plB[b], NB, 0, GS3), 1.0)
                vector.memset(pview(plB[b], NB, 1, GS3), 0.0)
            for s in range(NSTRIPS3):
                c, o, sz = STRIPS3[s]
                b = s % 2
                vector.wait_ge(act_xh, s + 1)
                if s >= 2:
                    vector.wait_ge(act_pair, 3 * (s - 1))  # args ring reuse
                    vector.wait_ge(pe_strip, s - 1)  # plane buffer reuse (sq)
                xin = xhv(c, o, sz)
                u = usc[:, 0:sz]
                r = rsc[:, 0:sz]
                for k, ts in enumerate((SCALE, 3 * SCALE, 9 * SCALE)):
                    t = slot(s, 2 * k, sz)
                    t2 = slot(s, 2 * k + 1, sz)
                    vector.tensor_scalar(u, xin, ts, None, OP.mult)
                    # r = round(u) via f16 magic add (output rounding), -1536
                    vector.tensor_scalar(r, u, 1536.0, None, OP.add)
                    vector.tensor_scalar(r, r, -1536.0, None, OP.add)
                    vector.tensor_sub(t, u, r)
                    # t2 = t - (t >= 0.25): phase for the cos plane
                    vector.tensor_scalar(r, t, 0.25, None, OP.is_ge)
                    vector.tensor_sub(t2, t, r).then_inc(dve_pair, 1)
                # derived planes: A2 = A1^2, B6 = B3^2 (sin stored halved)
                tqv = sqt.ap().rearrange("p (blk g) -> p blk g", g=16)[
                    :, 0 : sz // 16, :
                ]
                for j, (pl_, NP) in enumerate([(plA[b], NA), (plB[b], NB)]):
                    vector.wait_ge(act_pair, 3 * s + j + 1)
                    cst = pview(pl_, NP, 2, sz)
                    sst = pview(pl_, NP, 3, sz)
                    vector.tensor_mul(tqv, sst, sst)
                    vector.tensor_scalar(
                        pview(pl_, NP, 4, sz), tqv, -2.0, 1.0, OP.mult, OP.add
                    )
                    vector.tensor_mul(pview(pl_, NP, 5, sz), cst, sst).then_inc(
                        dve_sq, 1
                    )
            # ---- tail: masked diagonal extraction from PSUM ----
            vector.wait_ge(pe_strip, NSTRIPS3)
            vector.wait_ge(dma_sem, 112)  # mask + sel consts loaded
            vector.tensor_mul(P3sb[:, :], ps[:, :], masksb[:, :])
            p3r = P3sb.ap().rearrange(
                "p (c qa m) -> p qa c m", c=CH_PER_CORE, m=MSG
            )
            r2v = R2sb.ap().rearrange("p (qa c) -> p qa c", c=CH_PER_CORE)
            vector.tensor_reduce(r2v, p3r, AX.X, OP.add).then_inc(dve_sem, 1)
            # S2 (psum) -> S3 (sbuf) once the selector matmul lands
            vector.wait_ge(pe_sem, 1)
            vector.tensor_copy(S3[:, :], S2[:, :]).then_inc(dve_sem, 1)
            # ---- post-gather: normalize + cumsum + loss (identical on all cores)
            # ph2 is [12, 2*64]: pred hists in cols 0:64, target in 64:128
            vector.wait_ge(pe_sem, 2)
            vector.tensor_reduce(
                hsum[:, :],
                ph2.ap().rearrange("p (h j) -> p h j", h=2),
                AX.X,
                OP.add,
            )
            vector.scalar_tensor_tensor(
                hsum[:, :], hsum[:, :], 1e-8, hsum[:, :], OP.add, OP.bypass
            )
            vector.reciprocal(rinv[:, :], hsum[:, :])
            for h in range(2):
                vector.tensor_tensor_scan(
                    cdfs[:, h * BINS : (h + 1) * BINS],
                    ph2[:, h * BINS : (h + 1) * BINS],
                    P3sb[0:12, 0:BINS],
                    0.0,
                    OP.add,
                    OP.bypass,
                )
            vector.scalar_tensor_tensor(
                t1s[:, :], cdfs[:, 0:BINS], rinv[:, 0:1], cdfs[:, 0:BINS],
                OP.mult, OP.bypass,
            )
            vector.scalar_tensor_tensor(
                t1s[:, :], cdfs[:, BINS : 2 * BINS], rinv[:, 1:2], t1s[:, :],
                OP.mult, OP.subtract,
            )
            vector.tensor_reduce(
                ra[:, :], t1s[:, :], AX.X, OP.add, apply_absolute_value=True
            ).then_inc(dve_sem, 1)

        @block.tensor
        def _(tensor: bass.BassTensorEngine):
            for s in range(NSTRIPS3):
                c, o, sz = STRIPS3[s]
                b = s % 2
                tensor.wait_ge(act_pair, 3 * (s + 1))
                tensor.wait_ge(dve_sq, 2 * (s + 1))
                nsg = sz // MSG
                for j in range(nsg):
                    ins = tensor.matmul(
                        ps[:, c * NA * MSG : (c + 1) * NA * MSG],
                        plB[b][:, j * MSG * NB : (j + 1) * MSG * NB],
                        plA[b][:, j * MSG * NA : (j + 1) * MSG * NA],
                        start=(o == 0 and j == 0),
                        stop=(o + sz == G and j == nsg - 1),
                    )
                ins.then_inc(pe_strip, 1)
            # selector matmul collapses the 16 diagonal blocks: S2 = SEL^T @ R2
            tensor.wait_ge(dve_sem, 1)
            tensor.matmul(
                S2[:, :], selsb[:, :], R2sb[:, :], start=True, stop=True
            ).then_inc(pe_sem, 1)
            # synthesis: pred channels -> ph2 cols 0:64, target -> 64:128
            tensor.wait_ge(dma_sem, 256)
            tensor.matmul(
                ph2[:, 0:BINS], SV[:, 0:12], k2sb[:, :], start=True, stop=True
            )
            tensor.matmul(
                ph2[:, BINS : 2 * BINS], SV[:, 12:24], k2sb[:, :],
                start=True, stop=True,
            ).then_inc(pe_sem, 1)
            tensor.wait_ge(dve_sem, 3)
            tensor.matmul(
                pl[0:1, 0:1], ones_sb[0:12, 0:1], ra[0:12, 0:1],
                start=True, stop=True,
            ).then_inc(pe_sem, 1)

        @block.gpsimd
        def _(gpsimd: bass.BassGpSimd):
            gpsimd.wait_ge(dma_sem, 128)
            gpsimd.collective_compute(
                "AllGather",
                OP.bypass,
                replica_groups=[list(range(N_CORES))],
                ins=[cdf_in.ap()],
                outs=[cdf_all.ap()],
            ).then_inc(cc_sem, 1)

    return nc


# ---------------- v2: 42-harmonic fallback (previous session) ----------------
T1G = 6
T2G = 7
NA2 = 2 * T2G
NB2 = 2 * T1G
BPAD = 16
PACK = BINS + 1


def _host_k_matrix():
    Pb = 64 * 1.25
    K = np.zeros((2 * T1G, BPAD, BINS), np.float64)
    j = np.arange(BINS)
    for t1 in range(T1G):
        for t2 in range(T2G):
            t = t1 * T2G + t2
            ct = (math.sqrt(2 * math.pi) / Pb) * math.exp(
                -0.5 * (2 * math.pi * t / Pb) ** 2
            )
            mult = 1.0 if t == 0 else 2.0
            w = mult * ct * np.exp(-1j * 2 * np.pi * t * (0.5 + j) / Pb)
            K[2 * t1, 2 * t2] = w.real
            K[2 * t1 + 1, 2 * t2 + 1] = -w.real
            K[2 * t1, 2 * t2 + 1] = -w.imag
            K[2 * t1 + 1, 2 * t2, :] = -w.imag
    K[9, :, :] *= 2.0
    K[:, 9, :] *= 2.0
    K[:, 13, :] *= 2.0
    consts = np.zeros((128, 129), np.float32)
    for p in range(128):
        for h in range(2):
            a = h * 8 + p // BPAD
            b = p % BPAD
            if a < 2 * T1G:
                consts[p, 1 + h * 64 : 1 + (h + 1) * 64] = K[a, b]
    return consts


GS = 1024
STRIPS = [(0, 0, 256), (0, 256, 256), (0, 512, 512), (0, 1024, 1024)] + [
    (1, 0, 1024), (1, 1024, 1024),
    (2, 0, 1024), (2, 1024, 512), (2, 1536, 512),
]
NSTRIPS = len(STRIPS)
NPAIR = 6


def build_nc_fourier():
    nc = bass.Bass(num_devices=N_CORES)

    x_ext = nc.declare_dram_parameter("x", [CH_PER_CORE, P, G], F32, isOutput=False)
    out_ext = nc.declare_dram_parameter("out", [1, 1], F32, isOutput=True)

    cdf_in = nc.dram_tensor("cdf_in", [1, CH_PER_CORE * PACK], F32)
    cdf_all = nc.dram_tensor(
        "cdf_all", [N_CORES, CH_PER_CORE * PACK], F32, addr_space="Shared"
    )
    sdram = nc.dram_tensor("sdram", [2 * T1G, CH_PER_CORE, BPAD], F32)
    consts_dram = nc.inline_tensor(_host_k_matrix(), name="consts_k")

    pairs = [
        (1, "A", 2),
        (T2G, "B", 2),
        (3, "A", 6),
        (3 * T2G, "B", 6),
        (5, "A", 10),
        (5 * T2G, "B", 10),
    ]
    squares = [
        ("A", 2, 4, False, 0),
        ("B", 2, 4, False, 1),
        ("A", 6, 12, True, 2),
        ("A", 4, 8, True, None),
        ("B", 4, 8, True, None),
    ]

    with ExitStack() as stack:
        e = stack.enter_context
        xs = e(nc.sbuf_tensor("xs", [P, CH_PER_CORE * G], F32))
        consts = e(nc.sbuf_tensor("consts", [P, 129], F32))
        plA = [e(nc.sbuf_tensor(f"plA{b}", [P, NA2 * GS], F16)) for b in range(2)]
        plB = [e(nc.sbuf_tensor(f"plB{b}", [P, NB2 * GS], F16)) for b in range(2)]
        args = e(nc.sbuf_tensor("args", [P, 8 * GS], F32))
        irnd = e(nc.sbuf_tensor("irnd", [P, 2 * GS], mybir.dt.int32))
        i1r = e(nc.sbuf_tensor("i1r", [P, 4 * GS], mybir.dt.int32))
        sqsc = e(nc.sbuf_tensor("sqsc", [P, 2 * GS], F16))
        ones_sb = e(nc.sbuf_tensor("ones", [P, 1], F32))
        S_sb = e(nc.sbuf_tensor("S_sb", [2 * T1G, CH_PER_CORE * BPAD], F32))
        svec = e(nc.sbuf_tensor("svec", [P, 2 * CH_PER_CORE], F32))
        hsum = e(nc.sbuf_tensor("hsum", [CH_PER_CORE, 1], F32))
        rinv = e(nc.sbuf_tensor("rinv", [CH_PER_CORE, 1], F32))
        packed = e(nc.sbuf_tensor("packed", [CH_PER_CORE, PACK], F32))
        gp = e(nc.sbuf_tensor("gp", [12, PACK], F32))
        gt = e(nc.sbuf_tensor("gt", [12, PACK], F32))
        t1s = e(nc.sbuf_tensor("t1s", [12, BINS], F32))
        t2s = e(nc.sbuf_tensor("t2s", [12, BINS], F32))
        ra = e(nc.sbuf_tensor("ra", [12, 1], F32))
        loss_sb = e(nc.sbuf_tensor("loss", [1, 1], F32))
        ps = e(nc.psum_tensor("ps", [2 * T1G, CH_PER_CORE * 2 * T2G], F32))
        ph2 = e(nc.psum_tensor("ph2", [CH_PER_CORE, BINS], F32))
        pl = e(nc.psum_tensor("pl", [1, 1], F32))
        dma_sem = e(nc.semaphore("dma_sem"))
        dve_pair = e(nc.semaphore("dve_pair"))
        act_pair = e(nc.semaphore("act_pair"))
        act_i1 = e(nc.semaphore("act_i1"))
        pe_strip = e(nc.semaphore("pe_strip"))
        dve_sq = e(nc.semaphore("dve_sq"))
        pe_sem = e(nc.semaphore("pe_sem"))
        dve_sem = e(nc.semaphore("dve_sem"))
        act_sem = e(nc.semaphore("act_sem"))
        cc_sem = e(nc.semaphore("cc_sem"))
        block = e(nc.Block())

        def strip_x(s):
            c, o, sz = STRIPS[s]
            return xs[:, c * G + o : c * G + o + sz]

        def pview(pl_, NP, q, sz):
            v = pl_.ap().rearrange("p (ghi q glo) -> p q ghi glo", q=NP, glo=64)
            return v[:, q, 0 : sz // 64, :]

        def blk(ap2d):
            return ap2d.rearrange("p (a b) -> p a b", b=64)

        @block.sync
        def _(sync: bass.BassEngine):
            sync.dma_start(out=consts[:, :], in_=consts_dram[:, :]).then_inc(
                dma_sem, 16
            )
            for c in range(CH_PER_CORE):
                sync.dma_start(
                    out=xs[:, c * G : (c + 1) * G], in_=x_ext[c, :, :]
                ).then_inc(dma_sem, 16)
            sync.wait_ge(dve_sem, 1)
            sync.dma_start(out=sdram[:, :, :], in_=S_sb[:, :]).then_inc(dma_sem, 16)
            for c in range(CH_PER_CORE):
                for h in range(2):
                    lo = h * 8
                    hi = min(2 * T1G, lo + 8)
                    sync.dma_start(
                        out=svec[0 : (hi - lo) * BPAD, 2 * c + h : 2 * c + h + 1],
                        in_=sdram[lo:hi, c, :],
                    ).then_inc(dma_sem, 16)
            sync.wait_ge(dve_sem, 2)
            sync.dma_start(out=cdf_in[:, :], in_=packed[:, :]).then_inc(dma_sem, 16)
            sync.wait_ge(cc_sem, 1)
            sync.dma_start(out=gp[:, :], in_=cdf_all[0:4, :]).then_inc(dma_sem, 16)
            sync.dma_start(out=gt[:, :], in_=cdf_all[4:8, :]).then_inc(dma_sem, 16)
            sync.wait_ge(act_sem, 1)
            sync.dma_start(out=out_ext[:, :], in_=loss_sb[:, :]).then_inc(dma_sem, 16)

        @block.vector
        def _(vector: bass.BassVectorEngine):
            vector.memset(ones_sb[:, :], 1.0)
            vector.memset(svec[:, :], 0.0)
            vector.memset(S_sb[:, :], 0.0)
            for b in range(2):
                vector.memset(pview(plA[b], NA2, 0, GS), 1.0)
                vector.memset(pview(plA[b], NA2, 1, GS), 0.0)
                vector.memset(pview(plB[b], NB2, 0, GS), 1.0)
                vector.memset(pview(plB[b], NB2, 1, GS), 0.0)
            vector.wait_ge(dma_sem, 64)
            for s in range(NSTRIPS):
                xin = strip_x(s)
                b = s % 2
                sz = STRIPS[s][2]

                def do_square(job, inc=False):
                    side, sq_, dq, halve, wk = job
                    pl_ = plA[b] if side == "A" else plB[b]
                    NP = NA2 if side == "A" else NB2
                    if wk is not None:
                        vector.wait_ge(act_pair, s * NPAIR + wk + 1)
                    sc = pview(pl_, NP, sq_, sz)
                    sn = pview(pl_, NP, sq_ + 1, sz)
                    dc = pview(pl_, NP, dq, sz)
                    ds = pview(pl_, NP, dq + 1, sz)
                    pair = pl_.ap().rearrange(
                        "p (ghi qq glo) -> p qq ghi glo", qq=NP // 2, glo=128
                    )[:, sq_ // 2, 0 : sz // 64, :]
                    sq2 = sqsc.ap().rearrange("p (a b) -> p a b", b=128)[
                        :, 0 : sz // 64, :
                    ]
                    vector.tensor_mul(sq2, pair, pair)
                    vector.tensor_sub(dc, sq2[:, :, 0:64], sq2[:, :, 64:128])
                    ins = vector.tensor_mul(ds, sc, sn)
                    if not halve:
                        ins = vector.tensor_scalar(ds, ds, 2.0, None, OP.mult)
                    if inc:
                        ins.then_inc(dve_sq, 1)

                for k, (t, _, _) in enumerate(pairs):
                    kg = s * NPAIR + k
                    if kg >= 4:
                        vector.wait_ge(act_pair, kg - 3)
                    vector.wait_ge(act_i1, kg + 1)
                    slot = (kg % 4) * 2
                    fr = args[:, slot * GS : slot * GS + sz]
                    frc = args[:, (slot + 1) * GS : (slot + 1) * GS + sz]
                    ir = irnd[:, (kg % 2) * GS : (kg % 2) * GS + sz]
                    i1 = i1r[:, (kg % 4) * GS : (kg % 4) * GS + sz]
                    vector.scalar_tensor_tensor(
                        fr, xin, t * 0.8, i1, OP.mult, OP.subtract
                    )
                    vector.tensor_scalar(ir, fr, 0.25, None, OP.add)
                    vector.scalar_tensor_tensor(
                        frc, fr, 0.25, ir, OP.add, OP.subtract
                    ).then_inc(dve_pair, 1)
                    if k == 2:
                        if s >= 2:
                            vector.wait_ge(pe_strip, s - 1)
                        do_square(squares[0])
                        do_square(squares[1])
                        do_square(squares[3])
                        do_square(squares[4])
                    elif k == 4:
                        do_square(squares[2], inc=True)
            vector.wait_ge(pe_strip, NSTRIPS)
            for c in range(CH_PER_CORE):
                ins = vector.tensor_copy(
                    S_sb[:, c * BPAD : c * BPAD + 2 * T2G],
                    ps[:, c * 2 * T2G : (c + 1) * 2 * T2G],
                )
            ins.then_inc(dve_sem, 1)
            vector.wait_ge(pe_sem, 1)
            vector.tensor_reduce(hsum[:, :], ph2[:, :], AX.X, OP.add)
            vector.scalar_tensor_tensor(
                hsum[:, :], hsum[:, :], 1e-8, hsum[:, :], OP.add, OP.bypass
            )
            vector.reciprocal(rinv[:, :], hsum[:, :])
            vector.tensor_tensor_scan(
                packed[:, 0:BINS],
                ph2[:, :],
                t1s[0:CH_PER_CORE, :],
                0.0,
                OP.add,
                OP.bypass,
            )
            vector.tensor_copy(packed[:, BINS : BINS + 1], rinv[:, :]).then_inc(
                dve_sem, 1
            )
            vector.wait_ge(dma_sem, 224)
            vector.scalar_tensor_tensor(
                t1s[:, :],
                gp[:, 0:BINS],
                gp[:, BINS : BINS + 1],
                gp[:, 0:BINS],
                OP.mult,
                OP.bypass,
            )
            vector.scalar_tensor_tensor(
                t2s[:, :],
                gt[:, 0:BINS],
                gt[:, BINS : BINS + 1],
                t1s[:, :],
                OP.mult,
                OP.subtract,
            )
            vector.tensor_reduce(
                ra[:, :], t2s[:, :], AX.X, OP.add, apply_absolute_value=True
            ).then_inc(dve_sem, 1)

        @block.scalar
        def _(scalar: bass.BassScalarEngine):
            scalar.wait_ge(dma_sem, 64)
            NKG = NSTRIPS * NPAIR
            all_pairs = [(s, k) + pairs[k] for s in range(NSTRIPS) for k in range(NPAIR)]

            def emit_i1(kg):
                s, k, t, _, _ = all_pairs[kg]
                sz = STRIPS[s][2]
                i1 = i1r[:, (kg % 4) * GS : (kg % 4) * GS + sz]
                return scalar.activation(
                    i1, strip_x(s), AF.Copy, bias=0.0, scale=t * 0.8
                ).then_inc(act_i1, 1)

            emit_i1(0)
            emit_i1(1)
            for kg in range(NKG):
                s, k, t, which, q = all_pairs[kg]
                b = s % 2
                if kg + 2 < NKG:
                    if kg >= 2:
                        scalar.wait_ge(dve_pair, kg - 1)
                    emit_i1(kg + 2)
                if k == 0 and s >= 2:
                    scalar.wait_ge(pe_strip, s - 1)
                scalar.wait_ge(dve_pair, kg + 1)
                sz = STRIPS[s][2]
                slot = (kg % 4) * 2
                fr = args[:, slot * GS : slot * GS + sz]
                frc = args[:, (slot + 1) * GS : (slot + 1) * GS + sz]
                pl_ = plA[b] if which == "A" else plB[b]
                NP = NA2 if which == "A" else NB2
                scalar.activation(
                    pview(pl_, NP, q, sz),
                    blk(frc),
                    AF.Sin,
                    bias=consts[:, 0:1],
                    scale=TWO_PI,
                )
                scalar.activation(
                    pview(pl_, NP, q + 1, sz),
                    blk(fr),
                    AF.Sin,
                    bias=consts[:, 0:1],
                    scale=TWO_PI,
                ).then_inc(act_pair, 1)
            scalar.wait_ge(pe_sem, 2)
            scalar.mul(loss_sb[:, :], pl[:, :], 1.0 / 768.0).then_inc(act_sem, 1)

        @block.tensor
        def _(tensor: bass.BassTensorEngine):
            for s in range(NSTRIPS):
                b = s % 2
                c, o, sz = STRIPS[s]
                tensor.wait_ge(act_pair, NPAIR * (s + 1))
                tensor.wait_ge(dve_sq, s + 1)
                for g in range(sz):
                    ghi, glo = divmod(g, 64)
                    bb = ghi * (NB2 * 64) + glo
                    aa = ghi * (NA2 * 64) + glo
                    ins = tensor.matmul(
                        ps[:, 2 * T2G * c : 2 * T2G * (c + 1)],
                        plB[b][:, bb : bb + (NB2 - 1) * 64 + 1 : 64],
                        plA[b][:, aa : aa + (NA2 - 1) * 64 + 1 : 64],
                        start=(o == 0 and g == 0),
                        stop=(o + sz == G and g == sz - 1),
                    )
                ins.then_inc(pe_strip, 1)
            tensor.wait_ge(dma_sem, 176)
            for h in range(2):
                ins = tensor.matmul(
                    ph2[:, :],
                    svec[:, h :: 2],
                    consts[:, 1 + h * BINS : 1 + (h + 1) * BINS],
                    start=(h == 0),
                    stop=(h == 1),
                )
            ins.then_inc(pe_sem, 1)
            tensor.wait_ge(dve_sem, 3)
            tensor.matmul(
                pl[0:1, 0:1], ones_sb[0:12, 0:1], ra[0:12, 0:1], start=True, stop=True
            ).then_inc(pe_sem, 1)

        @block.gpsimd
        def _(gpsimd: bass.BassGpSimd):
            gpsimd.wait_ge(dma_sem, 192)
            gpsimd.collective_compute(
                "AllGather",
                OP.bypass,
                replica_groups=[list(range(N_CORES))],
                ins=[cdf_in.ap()],
                outs=[cdf_all.ap()],
            ).then_inc(cc_sem, 1)

    return nc


_VERSION = int(os.environ.get("BASS_HIST_V", "3"))
_NC_CACHE = None


def _get_nc():
    global _NC_CACHE
    if _NC_CACHE is None:
        _NC_CACHE = build_nc_v3() if _VERSION == 3 else build_nc_fourier()
    return _NC_CACHE


def kernel(pred: np.ndarray, target: np.ndarray) -> np.ndarray:
    assert pred.shape == (4, 3, 512, 512) and target.shape == (4, 3, 512, 512)
    chans = np.concatenate(
        [
            np.ascontiguousarray(pred, dtype=np.float32).reshape(12, P, G),
            np.ascontiguousarray(target, dtype=np.float32).reshape(12, P, G),
        ],
        axis=0,
    )  # [24, 128, 2048]

    in_maps = [
        {"x": np.ascontiguousarray(chans[3 * i : 3 * i + 3])} for i in range(N_CORES)
    ]

    nc = _get_nc()
    last_err = None
    for _attempt in range(3):
        try:
            res = run_bass_kernel_spmd(nc, in_maps, core_ids=list(range(N_CORES)))
            out = res.results[0]["out"]
            return np.asarray(out, dtype=np.float32).reshape(())
        except Exception as err:  # transient NRT_EXEC_UNIT_UNRECOVERABLE retries
            last_err = err
    raise last_err


# revision 39
# speedup vs baseline: 2.1924x; 1.0852x over previous
"""ColorHistogramLoss Trainium2 kernel (8 NeuronCores, SPMD).

Sharding: 24 channels total (pred 12 + target 12, channel = (tensor,b,c));
core i owns channels {3i, 3i+1, 3i+2}, each laid out [128, 2048] f32.

v3 (default) - 12-harmonic Fourier factorization, period Pb=70 bins.
The soft histogram hist_j = sum_p exp(-(64 x_p - j - 0.5)^2 / 2) is
approximated from harmonics t = 3*t1 + t2 (t1<4, t2<3) of the periodized
Gaussian: S(t1,t2) = sum_p B_t1(x_p) A_t2(x_p), a bilinear form computed by
the PE in block-diagonal supergroups of M=16 pixel-groups per LDW+MATMUL
(out [8*16, 6*16], useful diagonal blocks accumulated in PSUM; 128
instruction pairs per channel instead of 2048).

Range reduction needs no int casts: the DVE tensor_scalar two-scalar form
fr0 = (x * t*64/70) mod 1 does it in one op; sin/cos come from ScalarE
Sin with args kept in [-pi, pi] via sign absorption into the synthesis
matrix K2 (stored planes are -sin / -cos). A2/B6 planes derive from
A1/B3 by f16 squaring (sin stored halved, compensated in K2); B9 is
direct trig from 3*fr0(B3) mod 1. All planes f16.

Each core AllGathers its raw S block (8x18 f32), then every core runs the
identical tail: one synthesis matmul for all 24 channels, normalize,
cumsum, loss. Simulated accuracy of this exact scheme: rel err ~3e-4.

v2 - previous 42-harmonic version (BASS_HIST_V=2), ~290-360us measured.
"""

import math
import os
from contextlib import ExitStack

import numpy as np

import concourse.bass as bass
import concourse.mybir as mybir
from concourse.bass_utils import run_bass_kernel_spmd

BINS = 64
N_CORES = 8
CH_PER_CORE = 3
P = 128
G = 2048  # 512*512 / 128
F32 = mybir.dt.float32
F16 = mybir.dt.float16
AX = mybir.AxisListType
OP = mybir.AluOpType
AF = mybir.ActivationFunctionType

# ---------------- v3: 12-harmonic scheme ----------------
PB = 70.0
SCALE = 64.0 / PB          # phase per unit x for t=1
NA = 6                     # A planes: (c0,s0,cA1,sA1,cA2,sA2h)
NB = 8                     # B planes: (c0,s0,cB3,sB3,cB6,sB6h,cB9,sB9)
MSG = 16                   # supergroup: groups per LDWEIGHTS+MATMUL
GS3 = 1024                 # plane buffer capacity (groups)
TWO_PI = 2.0 * math.pi
# per-channel strips (offset, size); sizes multiples of 16
CH_STRIPS = [(0, 1024), (1024, 1024)]
STRIPS3 = [(c, o, sz) for c in range(CH_PER_CORE) for (o, sz) in CH_STRIPS]
NSTRIPS3 = len(STRIPS3)
# dma_sem threshold for each strip's x data (ch0a=16, ch0b=32, ch1=48, ch2=64)
def _strip_dma_need(c, o):
    if c == 0:
        return 16 if o == 0 else 32
    return 48 if c == 1 else 64


def _host_k2():
    """Synthesis matrix K2 [48, 64]: row (qb*6+qa), col j.

    hist_j = sum_{t1,t2} Re[w(t) * S(t1,t2)] where S is the bilinear sum of
    TRUE plane values. Stored planes are affine transforms of the true
    values: true = a*stored + b. The b-terms are folded onto the const
    (q0) plane rows. Per plane (a, b):
      q0 const-cos: (1, 0) [value 1]; q1 const-sin: (1, 0) [value 0]
      q2/q3 trig cos/sin: (1, 0)
      q4 derived cos: stored = sin^2(half) -> true = 1 - 2*stored: (-2, 1)
      q5 derived sin: stored = cos*sin = sin(2x)/2 -> (2, 0)
    """
    jj = np.arange(BINS)
    abA = [[(1.0, 0.0)] * 2, [(1.0, 0.0)] * 2, [(-2.0, 1.0), (2.0, 0.0)]]
    abB = [
        [(1.0, 0.0)] * 2,
        [(1.0, 0.0)] * 2,
        [(-2.0, 1.0), (2.0, 0.0)],
        [(1.0, 0.0)] * 2,
    ]
    K = np.zeros((NB, NA, BINS))
    for t1 in range(4):
        for t2 in range(3):
            t = 3 * t1 + t2
            ct = (math.sqrt(2 * math.pi) / PB) * math.exp(
                -0.5 * (2 * math.pi * t / PB) ** 2
            )
            mult = 1.0 if t == 0 else 2.0
            w = mult * ct * np.exp(-1j * 2 * np.pi * t * (0.5 + jj) / PB)
            # coefficients on TRUE products: (cB,cA): Re w; (sB,sA): -Re w;
            # (sB,cA): -Im w; (cB,sA): -Im w
            for (ib, ia), coef in [
                ((0, 0), w.real),
                ((1, 1), -w.real),
                ((1, 0), -w.imag),
                ((0, 1), -w.imag),
            ]:
                qb, qa = 2 * t1 + ib, 2 * t2 + ia
                aB, bB = abB[t1][ib]
                aA, bA = abA[t2][ia]
                K[qb, qa] += coef * aB * aA
                K[qb, 0] += coef * aB * bA
                K[0, qa] += coef * bB * aA
                K[0, 0] += coef * bB * bA
    return np.ascontiguousarray(K.reshape(NB * NA, BINS).astype(np.float32))


def _host_mask():
    """Diagonal mask [128, 288]: 1 where psum col's m' == partition's m."""
    m = np.zeros((P, CH_PER_CORE * NA * MSG), np.float32)
    for p in range(P):
        for col in range(CH_PER_CORE * NA * MSG):
            if col % MSG == p % MSG:
                m[p, col] = 1.0
    return m


def _host_sel():
    """Block selector [128, 8]: SEL[p, qb] = 1 iff p // 16 == qb."""
    s = np.zeros((P, NB), np.float32)
    for p in range(P):
        s[p, p // MSG] = 1.0
    return s


def build_nc_v3():
    nc = bass.Bass(num_devices=N_CORES)

    x_ext = nc.declare_dram_parameter("x", [CH_PER_CORE, P, G], F32, isOutput=False)
    out_ext = nc.declare_dram_parameter("out", [1, 1], F32, isOutput=True)

    cdf_in = nc.dram_tensor("cdf_in", [NB, CH_PER_CORE * NA], F32)
    cdf_all = nc.dram_tensor(
        "cdf_all", [N_CORES * NB, CH_PER_CORE * NA], F32, addr_space="Shared"
    )
    k2_dram = nc.inline_tensor(_host_k2(), name="k2_const")
    mask_dram = nc.inline_tensor(_host_mask(), name="mask_const")
    sel_dram = nc.inline_tensor(_host_sel(), name="sel_const")

    with ExitStack() as stack:
        e = stack.enter_context
        xs = e(nc.sbuf_tensor("xs", [P, CH_PER_CORE * G], F32))
        xh = e(nc.sbuf_tensor("xh", [P, CH_PER_CORE * G], F16))
        # 16-interleaved layout: col = blk*(NP*16) + q*16 + (g%16)
        plA = [e(nc.sbuf_tensor(f"plA{b}", [P, GS3 * NA], F16)) for b in range(2)]
        plB = [e(nc.sbuf_tensor(f"plB{b}", [P, GS3 * NB], F16)) for b in range(2)]
        # args ring: 6 slots/strip (t, t2 for the 3 trig pairs), 2 bufs
        args = e(nc.sbuf_tensor("args", [P, 12 * GS3], F16))
        usc = e(nc.sbuf_tensor("usc", [P, GS3], F16))
        rsc = e(nc.sbuf_tensor("rsc", [P, GS3], F16))
        P3sb = e(nc.sbuf_tensor("P3sb", [P, CH_PER_CORE * NA * MSG], F32))
        R2sb = e(nc.sbuf_tensor("R2sb", [P, CH_PER_CORE * NA], F32))
        masksb = e(nc.sbuf_tensor("masksb", [P, CH_PER_CORE * NA * MSG], F32))
        selsb = e(nc.sbuf_tensor("selsb", [P, NB], F32))
        S3 = e(nc.sbuf_tensor("S3", [NB, CH_PER_CORE * NA], F32))
        SV = e(nc.sbuf_tensor("SV", [NB * NA, N_CORES * CH_PER_CORE], F32))
        k2sb = e(nc.sbuf_tensor("k2sb", [NB * NA, BINS], F32))
        cdfs = e(nc.sbuf_tensor("cdfs", [12, 2 * BINS], F32))
        hsum = e(nc.sbuf_tensor("hsum", [12, 2], F32))
        rinv = e(nc.sbuf_tensor("rinv", [12, 2], F32))
        t1s = e(nc.sbuf_tensor("t1s", [12, BINS], F32))
        ra = e(nc.sbuf_tensor("ra", [12, 1], F32))
        ones_sb = e(nc.sbuf_tensor("ones", [P, 1], F32))
        pospi2 = e(nc.sbuf_tensor("pospi2", [P, 1], F32))
        zerob = e(nc.sbuf_tensor("zerob", [P, 1], F32))
        loss_sb = e(nc.sbuf_tensor("loss", [1, 1], F32))
        ps = e(nc.psum_tensor("ps", [P, CH_PER_CORE * NA * MSG], F32))
        S2 = e(nc.psum_tensor("S2", [NB, CH_PER_CORE * NA], F32))
        ph2 = e(nc.psum_tensor("ph2", [12, 2 * BINS], F32))
        pl = e(nc.psum_tensor("pl", [1, 1], F32))
        dma_sem = e(nc.semaphore("dma_sem"))
        act_xh = e(nc.semaphore("act_xh"))
        dve_pair = e(nc.semaphore("dve_pair"))
        act_pair = e(nc.semaphore("act_pair"))
        dve_sq = e(nc.semaphore("dve_sq"))
        pe_strip = e(nc.semaphore("pe_strip"))
        dve_sem = e(nc.semaphore("dve_sem"))
        pe_sem = e(nc.semaphore("pe_sem"))
        act_sem = e(nc.semaphore("act_sem"))
        cc_sem = e(nc.semaphore("cc_sem"))
        block = e(nc.Block())

        def xv(c, o, sz):
            return xs[:, c * G + o : c * G + o + sz]

        def xhv(c, o, sz):
            return xh[:, c * G + o : c * G + o + sz]

        def pview(pl_, NP, q, sz):
            # plane q over the strip: [p, sz/16, 16] (16-contiguous runs)
            v = pl_.ap().rearrange("p (blk q g) -> p q blk g", q=NP, g=16)
            return v[:, q, 0 : sz // 16, :]

        def slot(s, k, sz):
            base = ((s % 2) * 6 + k) * GS3
            return args[:, base : base + sz]

        @block.sync
        def _(sync: bass.BassEngine):
            sync.dma_start(out=xs[:, 0:1024], in_=x_ext[0, :, 0:1024]).then_inc(
                dma_sem, 16
            )
            sync.dma_start(out=xs[:, 1024:G], in_=x_ext[0, :, 1024:G]).then_inc(
                dma_sem, 16
            )
            sync.dma_start(out=xs[:, G : 2 * G], in_=x_ext[1, :, :]).then_inc(
                dma_sem, 16
            )
            sync.dma_start(out=xs[:, 2 * G : 3 * G], in_=x_ext[2, :, :]).then_inc(
                dma_sem, 16
            )
            sync.dma_start(out=k2sb[:, :], in_=k2_dram[:, :]).then_inc(dma_sem, 16)
            sync.dma_start(out=masksb[:, :], in_=mask_dram[:, :]).then_inc(
                dma_sem, 16
            )
            sync.dma_start(out=selsb[:, :], in_=sel_dram[:, :]).then_inc(dma_sem, 16)
            # raw S block -> collective input
            sync.wait_ge(dve_sem, 2)
            sync.dma_start(out=cdf_in[:, :], in_=S3[0:NB, :]).then_inc(dma_sem, 16)
            # gathered S -> SV [48, 24]: SV[qb*6+qa, core*3+c] = cdf_all[core*8+qb, qa*3+c]
            sync.wait_ge(cc_sem, 1)
            gat = cdf_all.ap().rearrange(
                "(core qb) (qa c) -> (qb qa) core c", qb=NB, c=CH_PER_CORE
            )
            sync.dma_start(out=SV[:, :], in_=gat).then_inc(dma_sem, 16)
            # final scalar
            sync.wait_ge(act_sem, 1)
            sync.dma_start(out=out_ext[:, :], in_=loss_sb[:, :]).then_inc(dma_sem, 16)

        @block.scalar
        def _(scalar: bass.BassScalarEngine):
            def emit_xh(s):
                c, o, sz = STRIPS3[s]
                scalar.wait_ge(dma_sem, _strip_dma_need(c, o))
                scalar.activation(xhv(c, o, sz), xv(c, o, sz), AF.Copy).then_inc(
                    act_xh, 1
                )

            emit_xh(0)
            for s in range(NSTRIPS3):
                c, o, sz = STRIPS3[s]
                b = s % 2
                if s + 1 < NSTRIPS3:
                    emit_xh(s + 1)
                if s >= 2:
                    scalar.wait_ge(pe_strip, s - 1)  # plane buffer reuse
                # trig pair k: cos -> plane q (from t2), sin -> q+1 (from t)
                for k, (pl_, NP, q) in enumerate(
                    [(plA[b], NA, 2), (plB[b], NB, 2), (plB[b], NB, 6)]
                ):
                    scalar.wait_ge(dve_pair, 3 * s + k + 1)
                    t = slot(s, 2 * k, sz)
                    t2 = slot(s, 2 * k + 1, sz)
                    tv = t.rearrange("p (blk g) -> p blk g", g=16)
                    t2v = t2.rearrange("p (blk g) -> p blk g", g=16)
                    scalar.activation(
                        pview(pl_, NP, q, sz), t2v, AF.Sin,
                        bias=pospi2[:, 0:1], scale=TWO_PI,
                    )
                    scalar.activation(
                        pview(pl_, NP, q + 1, sz), tv, AF.Sin,
                        bias=zerob[:, 0:1], scale=TWO_PI,
                    ).then_inc(act_pair, 1)
            # final: loss = pl / 768
            scalar.wait_ge(pe_sem, 3)
            scalar.mul(loss_sb[:, :], pl[:, :], 1.0 / 768.0).then_inc(act_sem, 1)

        @block.vector
        def _(vector: bass.BassVectorEngine):
            vector.memset(ones_sb[:, :], 1.0)
            vector.memset(pospi2[:, :], math.pi / 2.0)
            vector.memset(zerob[:, :], 0.0)
            for b in range(2):
                vector.memset(pview(plA[b], NA, 0, GS3), 1.0)
                vector.memset(pview(plA[b], NA, 1, GS3), 0.0)
                vector.memset(pview(plB[b], NB, 0, GS3), 1.0)
                vector.memset(pview(plB[b], NB, 1, GS3), 0.0)
            for s in range(NSTRIPS3):
                c, o, sz = STRIPS3[s]
                b = s % 2
                vector.wait_ge(act_xh, s + 1)
                if s >= 2:
                    vector.wait_ge(act_pair, 3 * (s - 1))  # args ring reuse
                    vector.wait_ge(pe_strip, s - 1)  # plane buffer reuse (sq)
                xin = xhv(c, o, sz)
                u = usc[:, 0:sz]
                r = rsc[:, 0:sz]
                for k, ts in enumerate((SCALE, 3 * SCALE, 9 * SCALE)):
                    t = slot(s, 2 * k, sz)
                    t2 = slot(s, 2 * k + 1, sz)
                    vector.tensor_scalar(u, xin, ts, None, OP.mult)
                    # r = round(u) via f16 magic add (output rounding), -1536
                    vector.tensor_scalar(r, u, 1536.0, None, OP.add)
                    vector.tensor_scalar(r, r, -1536.0, None, OP.add)
                    vector.tensor_sub(t, u, r)
                    # t2 = t - (t >= 0.25): phase for the cos plane
                    vector.tensor_scalar(r, t, 0.25, None, OP.is_ge)
                    vector.tensor_sub(t2, t, r).then_inc(dve_pair, 1)
                # derived planes: store s^2 (q4) and c*s (q5) raw; the
                # 1-2s^2 / doubling transforms are folded into K2
                for j, (pl_, NP) in enumerate([(plA[b], NA), (plB[b], NB)]):
                    vector.wait_ge(act_pair, 3 * s + j + 1)
                    cst = pview(pl_, NP, 2, sz)
                    sst = pview(pl_, NP, 3, sz)
                    vector.tensor_mul(pview(pl_, NP, 4, sz), sst, sst)
                    vector.tensor_mul(pview(pl_, NP, 5, sz), cst, sst).then_inc(
                        dve_sq, 1
                    )
            # ---- tail: masked diagonal extraction from PSUM ----
            vector.wait_ge(pe_strip, NSTRIPS3)
            vector.wait_ge(dma_sem, 112)  # mask + sel consts loaded
            vector.tensor_mul(P3sb[:, :], ps[:, :], masksb[:, :])
            p3r = P3sb.ap().rearrange(
                "p (c qa m) -> p qa c m", c=CH_PER_CORE, m=MSG
            )
            r2v = R2sb.ap().rearrange("p (qa c) -> p qa c", c=CH_PER_CORE)
            vector.tensor_reduce(r2v, p3r, AX.X, OP.add).then_inc(dve_sem, 1)
            # S2 (psum) -> S3 (sbuf) once the selector matmul lands
            vector.wait_ge(pe_sem, 1)
            vector.tensor_copy(S3[:, :], S2[:, :]).then_inc(dve_sem, 1)
            # ---- post-gather: normalize + cumsum + loss (identical on all cores)
            # ph2 is [12, 2*64]: pred hists in cols 0:64, target in 64:128
            vector.wait_ge(pe_sem, 2)
            vector.tensor_reduce(
                hsum[:, :],
                ph2.ap().rearrange("p (h j) -> p h j", h=2),
                AX.X,
                OP.add,
            )
            vector.scalar_tensor_tensor(
                hsum[:, :], hsum[:, :], 1e-8, hsum[:, :], OP.add, OP.bypass
            )
            vector.reciprocal(rinv[:, :], hsum[:, :])
            for h in range(2):
                vector.tensor_tensor_scan(
                    cdfs[:, h * BINS : (h + 1) * BINS],
                    ph2[:, h * BINS : (h + 1) * BINS],
                    P3sb[0:12, 0:BINS],
                    0.0,
                    OP.add,
                    OP.bypass,
                )
            vector.scalar_tensor_tensor(
                t1s[:, :], cdfs[:, 0:BINS], rinv[:, 0:1], cdfs[:, 0:BINS],
                OP.mult, OP.bypass,
            )
            vector.scalar_tensor_tensor(
                t1s[:, :], cdfs[:, BINS : 2 * BINS], rinv[:, 1:2], t1s[:, :],
                OP.mult, OP.subtract,
            )
            vector.tensor_reduce(
                ra[:, :], t1s[:, :], AX.X, OP.add, apply_absolute_value=True
            ).then_inc(dve_sem, 1)

        @block.tensor
        def _(tensor: bass.BassTensorEngine):
            for s in range(NSTRIPS3):
                c, o, sz = STRIPS3[s]
                b = s % 2
                tensor.wait_ge(act_pair, 3 * (s + 1))
                tensor.wait_ge(dve_sq, 2 * (s + 1))
                nsg = sz // MSG
                for j in range(nsg):
                    ins = tensor.matmul(
                        ps[:, c * NA * MSG : (c + 1) * NA * MSG],
                        plB[b][:, j * MSG * NB : (j + 1) * MSG * NB],
                        plA[b][:, j * MSG * NA : (j + 1) * MSG * NA],
                        start=(o == 0 and j == 0),
                        stop=(o + sz == G and j == nsg - 1),
                    )
                ins.then_inc(pe_strip, 1)
            # selector matmul collapses the 16 diagonal blocks: S2 = SEL^T @ R2
            tensor.wait_ge(dve_sem, 1)
            tensor.matmul(
                S2[:, :], selsb[:, :], R2sb[:, :], start=True, stop=True
            ).then_inc(pe_sem, 1)
            # synthesis: pred channels -> ph2 cols 0:64, target -> 64:128
            tensor.wait_ge(dma_sem, 144)
            tensor.matmul(
                ph2[:, 0:BINS], SV[:, 0:12], k2sb[:, :], start=True, stop=True
            )
            tensor.matmul(
                ph2[:, BINS : 2 * BINS], SV[:, 12:24], k2sb[:, :],
                start=True, stop=True,
            ).then_inc(pe_sem, 1)
            tensor.wait_ge(dve_sem, 3)
            tensor.matmul(
                pl[0:1, 0:1], ones_sb[0:12, 0:1], ra[0:12, 0:1],
                start=True, stop=True,
            ).then_inc(pe_sem, 1)

        @block.gpsimd
        def _(gpsimd: bass.BassGpSimd):
            gpsimd.wait_ge(dma_sem, 128)
            gpsimd.collective_compute(
                "AllGather",
                OP.bypass,
                replica_groups=[list(range(N_CORES))],
                ins=[cdf_in.ap()],
                outs=[cdf_all.ap()],
            ).then_inc(cc_sem, 1)

    return nc


# ---------------- v2: 42-harmonic fallback (previous session) ----------------
T1G = 6
T2G = 7
NA2 = 2 * T2G
NB2 = 2 * T1G
BPAD = 16
PACK = BINS + 1


def _host_k_matrix():
    Pb = 64 * 1.25
    K = np.zeros((2 * T1G, BPAD, BINS), np.float64)
    j = np.arange(BINS)
    for t1 in range(T1G):
        for t2 in range(T2G):
            t = t1 * T2G + t2
            ct = (math.sqrt(2 * math.pi) / Pb) * math.exp(
                -0.5 * (2 * math.pi * t / Pb) ** 2
            )
            mult = 1.0 if t == 0 else 2.0
            w = mult * ct * np.exp(-1j * 2 * np.pi * t * (0.5 + j) / Pb)
            K[2 * t1, 2 * t2] = w.real
            K[2 * t1 + 1, 2 * t2 + 1] = -w.real
            K[2 * t1, 2 * t2 + 1] = -w.imag
            K[2 * t1 + 1, 2 * t2, :] = -w.imag
    K[9, :, :] *= 2.0
    K[:, 9, :] *= 2.0
    K[:, 13, :] *= 2.0
    consts = np.zeros((128, 129), np.float32)
    for p in range(128):
        for h in range(2):
            a = h * 8 + p // BPAD
            b = p % BPAD
            if a < 2 * T1G:
                consts[p, 1 + h * 64 : 1 + (h + 1) * 64] = K[a, b]
    return consts


GS = 1024
STRIPS = [(0, 0, 256), (0, 256, 256), (0, 512, 512), (0, 1024, 1024)] + [
    (1, 0, 1024), (1, 1024, 1024),
    (2, 0, 1024), (2, 1024, 512), (2, 1536, 512),
]
NSTRIPS = len(STRIPS)
NPAIR = 6


def build_nc_fourier():
    nc = bass.Bass(num_devices=N_CORES)

    x_ext = nc.declare_dram_parameter("x", [CH_PER_CORE, P, G], F32, isOutput=False)
    out_ext = nc.declare_dram_parameter("out", [1, 1], F32, isOutput=True)

    cdf_in = nc.dram_tensor("cdf_in", [1, CH_PER_CORE * PACK], F32)
    cdf_all = nc.dram_tensor(
        "cdf_all", [N_CORES, CH_PER_CORE * PACK], F32, addr_space="Shared"
    )
    sdram = nc.dram_tensor("sdram", [2 * T1G, CH_PER_CORE, BPAD], F32)
    consts_dram = nc.inline_tensor(_host_k_matrix(), name="consts_k")

    pairs = [
        (1, "A", 2),
        (T2G, "B", 2),
        (3, "A", 6),
        (3 * T2G, "B", 6),
        (5, "A", 10),
        (5 * T2G, "B", 10),
    ]
    squares = [
        ("A", 2, 4, False, 0),
        ("B", 2, 4, False, 1),
        ("A", 6, 12, True, 2),
        ("A", 4, 8, True, None),
        ("B", 4, 8, True, None),
    ]

    with ExitStack() as stack:
        e = stack.enter_context
        xs = e(nc.sbuf_tensor("xs", [P, CH_PER_CORE * G], F32))
        consts = e(nc.sbuf_tensor("consts", [P, 129], F32))
        plA = [e(nc.sbuf_tensor(f"plA{b}", [P, NA2 * GS], F16)) for b in range(2)]
        plB = [e(nc.sbuf_tensor(f"plB{b}", [P, NB2 * GS], F16)) for b in range(2)]
        args = e(nc.sbuf_tensor("args", [P, 8 * GS], F32))
        irnd = e(nc.sbuf_tensor("irnd", [P, 2 * GS], mybir.dt.int32))
        i1r = e(nc.sbuf_tensor("i1r", [P, 4 * GS], mybir.dt.int32))
        sqsc = e(nc.sbuf_tensor("sqsc", [P, 2 * GS], F16))
        ones_sb = e(nc.sbuf_tensor("ones", [P, 1], F32))
        S_sb = e(nc.sbuf_tensor("S_sb", [2 * T1G, CH_PER_CORE * BPAD], F32))
        svec = e(nc.sbuf_tensor("svec", [P, 2 * CH_PER_CORE], F32))
        hsum = e(nc.sbuf_tensor("hsum", [CH_PER_CORE, 1], F32))
        rinv = e(nc.sbuf_tensor("rinv", [CH_PER_CORE, 1], F32))
        packed = e(nc.sbuf_tensor("packed", [CH_PER_CORE, PACK], F32))
        gp = e(nc.sbuf_tensor("gp", [12, PACK], F32))
        gt = e(nc.sbuf_tensor("gt", [12, PACK], F32))
        t1s = e(nc.sbuf_tensor("t1s", [12, BINS], F32))
        t2s = e(nc.sbuf_tensor("t2s", [12, BINS], F32))
        ra = e(nc.sbuf_tensor("ra", [12, 1], F32))
        loss_sb = e(nc.sbuf_tensor("loss", [1, 1], F32))
        ps = e(nc.psum_tensor("ps", [2 * T1G, CH_PER_CORE * 2 * T2G], F32))
        ph2 = e(nc.psum_tensor("ph2", [CH_PER_CORE, BINS], F32))
        pl = e(nc.psum_tensor("pl", [1, 1], F32))
        dma_sem = e(nc.semaphore("dma_sem"))
        dve_pair = e(nc.semaphore("dve_pair"))
        act_pair = e(nc.semaphore("act_pair"))
        act_i1 = e(nc.semaphore("act_i1"))
        pe_strip = e(nc.semaphore("pe_strip"))
        dve_sq = e(nc.semaphore("dve_sq"))
        pe_sem = e(nc.semaphore("pe_sem"))
        dve_sem = e(nc.semaphore("dve_sem"))
        act_sem = e(nc.semaphore("act_sem"))
        cc_sem = e(nc.semaphore("cc_sem"))
        block = e(nc.Block())

        def strip_x(s):
            c, o, sz = STRIPS[s]
            return xs[:, c * G + o : c * G + o + sz]

        def pview(pl_, NP, q, sz):
            v = pl_.ap().rearrange("p (ghi q glo) -> p q ghi glo", q=NP, glo=64)
            return v[:, q, 0 : sz // 64, :]

        def blk(ap2d):
            return ap2d.rearrange("p (a b) -> p a b", b=64)

        @block.sync
        def _(sync: bass.BassEngine):
            sync.dma_start(out=consts[:, :], in_=consts_dram[:, :]).then_inc(
                dma_sem, 16
            )
            for c in range(CH_PER_CORE):
                sync.dma_start(
                    out=xs[:, c * G : (c + 1) * G], in_=x_ext[c, :, :]
                ).then_inc(dma_sem, 16)
            sync.wait_ge(dve_sem, 1)
            sync.dma_start(out=sdram[:, :, :], in_=S_sb[:, :]).then_inc(dma_sem, 16)
            for c in range(CH_PER_CORE):
                for h in range(2):
                    lo = h * 8
                    hi = min(2 * T1G, lo + 8)
                    sync.dma_start(
                        out=svec[0 : (hi - lo) * BPAD, 2 * c + h : 2 * c + h + 1],
                        in_=sdram[lo:hi, c, :],
                    ).then_inc(dma_sem, 16)
            sync.wait_ge(dve_sem, 2)
            sync.dma_start(out=cdf_in[:, :], in_=packed[:, :]).then_inc(dma_sem, 16)
            sync.wait_ge(cc_sem, 1)
            sync.dma_start(out=gp[:, :], in_=cdf_all[0:4, :]).then_inc(dma_sem, 16)
            sync.dma_start(out=gt[:, :], in_=cdf_all[4:8, :]).then_inc(dma_sem, 16)
            sync.wait_ge(act_sem, 1)
            sync.dma_start(out=out_ext[:, :], in_=loss_sb[:, :]).then_inc(dma_sem, 16)

        @block.vector
        def _(vector: bass.BassVectorEngine):
            vector.memset(ones_sb[:, :], 1.0)
            vector.memset(svec[:, :], 0.0)
            vector.memset(S_sb[:, :], 0.0)
            for b in range(2):
                vector.memset(pview(plA[b], NA2, 0, GS), 1.0)
                vector.memset(pview(plA[b], NA2, 1, GS), 0.0)
                vector.memset(pview(plB[b], NB2, 0, GS), 1.0)
                vector.memset(pview(plB[b], NB2, 1, GS), 0.0)
            vector.wait_ge(dma_sem, 64)
            for s in range(NSTRIPS):
                xin = strip_x(s)
                b = s % 2
                sz = STRIPS[s][2]

                def do_square(job, inc=False):
                    side, sq_, dq, halve, wk = job
                    pl_ = plA[b] if side == "A" else plB[b]
                    NP = NA2 if side == "A" else NB2
                    if wk is not None:
                        vector.wait_ge(act_pair, s * NPAIR + wk + 1)
                    sc = pview(pl_, NP, sq_, sz)
                    sn = pview(pl_, NP, sq_ + 1, sz)
                    dc = pview(pl_, NP, dq, sz)
                    ds = pview(pl_, NP, dq + 1, sz)
                    pair = pl_.ap().rearrange(
                        "p (ghi qq glo) -> p qq ghi glo", qq=NP // 2, glo=128
                    )[:, sq_ // 2, 0 : sz // 64, :]
                    sq2 = sqsc.ap().rearrange("p (a b) -> p a b", b=128)[
                        :, 0 : sz // 64, :
                    ]
                    vector.tensor_mul(sq2, pair, pair)
                    vector.tensor_sub(dc, sq2[:, :, 0:64], sq2[:, :, 64:128])
                    ins = vector.tensor_mul(ds, sc, sn)
                    if not halve:
                        ins = vector.tensor_scalar(ds, ds, 2.0, None, OP.mult)
                    if inc:
                        ins.then_inc(dve_sq, 1)

                for k, (t, _, _) in enumerate(pairs):
                    kg = s * NPAIR + k
                    if kg >= 4:
                        vector.wait_ge(act_pair, kg - 3)
                    vector.wait_ge(act_i1, kg + 1)
                    slot = (kg % 4) * 2
                    fr = args[:, slot * GS : slot * GS + sz]
                    frc = args[:, (slot + 1) * GS : (slot + 1) * GS + sz]
                    ir = irnd[:, (kg % 2) * GS : (kg % 2) * GS + sz]
                    i1 = i1r[:, (kg % 4) * GS : (kg % 4) * GS + sz]
                    vector.scalar_tensor_tensor(
                        fr, xin, t * 0.8, i1, OP.mult, OP.subtract
                    )
                    vector.tensor_scalar(ir, fr, 0.25, None, OP.add)
                    vector.scalar_tensor_tensor(
                        frc, fr, 0.25, ir, OP.add, OP.subtract
                    ).then_inc(dve_pair, 1)
                    if k == 2:
                        if s >= 2:
                            vector.wait_ge(pe_strip, s - 1)
                        do_square(squares[0])
                        do_square(squares[1])
                        do_square(squares[3])
                        do_square(squares[4])
                    elif k == 4:
                        do_square(squares[2], inc=True)
            vector.wait_ge(pe_strip, NSTRIPS)
            for c in range(CH_PER_CORE):
                ins = vector.tensor_copy(
                    S_sb[:, c * BPAD : c * BPAD + 2 * T2G],
                    ps[:, c * 2 * T2G : (c + 1) * 2 * T2G],
                )
            ins.then_inc(dve_sem, 1)
            vector.wait_ge(pe_sem, 1)
            vector.tensor_reduce(hsum[:, :], ph2[:, :], AX.X, OP.add)
            vector.scalar_tensor_tensor(
                hsum[:, :], hsum[:, :], 1e-8, hsum[:, :], OP.add, OP.bypass
            )
            vector.reciprocal(rinv[:, :], hsum[:, :])
            vector.tensor_tensor_scan(
                packed[:, 0:BINS],
                ph2[:, :],
                t1s[0:CH_PER_CORE, :],
                0.0,
                OP.add,
                OP.bypass,
            )
            vector.tensor_copy(packed[:, BINS : BINS + 1], rinv[:, :]).then_inc(
                dve_sem, 1
            )
            vector.wait_ge(dma_sem, 224)
            vector.scalar_tensor_tensor(
                t1s[:, :],
                gp[:, 0:BINS],
                gp[:, BINS : BINS + 1],
                gp[:, 0:BINS],
                OP.mult,
                OP.bypass,
            )
            vector.scalar_tensor_tensor(
                t2s[:, :],
                gt[:, 0:BINS],
                gt[:, BINS : BINS + 1],
                t1s[:, :],
                OP.mult,
                OP.subtract,
            )
            vector.tensor_reduce(
                ra[:, :], t2s[:, :], AX.X, OP.add, apply_absolute_value=True
            ).then_inc(dve_sem, 1)

        @block.scalar
        def _(scalar: bass.BassScalarEngine):
            scalar.wait_ge(dma_sem, 64)
            NKG = NSTRIPS * NPAIR
            all_pairs = [(s, k) + pairs[k] for s in range(NSTRIPS) for k in range(NPAIR)]

            def emit_i1(kg):
                s, k, t, _, _ = all_pairs[kg]
                sz = STRIPS[s][2]
                i1 = i1r[:, (kg % 4) * GS : (kg % 4) * GS + sz]
                return scalar.activation(
                    i1, strip_x(s), AF.Copy, bias=0.0, scale=t * 0.8
                ).then_inc(act_i1, 1)

            emit_i1(0)
            emit_i1(1)
            for kg in range(NKG):
                s, k, t, which, q = all_pairs[kg]
                b = s % 2
                if kg + 2 < NKG:
                    if kg >= 2:
                        scalar.wait_ge(dve_pair, kg - 1)
                    emit_i1(kg + 2)
                if k == 0 and s >= 2:
                    scalar.wait_ge(pe_strip, s - 1)
                scalar.wait_ge(dve_pair, kg + 1)
                sz = STRIPS[s][2]
                slot = (kg % 4) * 2
                fr = args[:, slot * GS : slot * GS + sz]
                frc = args[:, (slot + 1) * GS : (slot + 1) * GS + sz]
                pl_ = plA[b] if which == "A" else plB[b]
                NP = NA2 if which == "A" else NB2
                scalar.activation(
                    pview(pl_, NP, q, sz),
                    blk(frc),
                    AF.Sin,
                    bias=consts[:, 0:1],
                    scale=TWO_PI,
                )
                scalar.activation(
                    pview(pl_, NP, q + 1, sz),
                    blk(fr),
                    AF.Sin,
                    bias=consts[:, 0:1],
                    scale=TWO_PI,
                ).then_inc(act_pair, 1)
            scalar.wait_ge(pe_sem, 2)
            scalar.mul(loss_sb[:, :], pl[:, :], 1.0 / 768.0).then_inc(act_sem, 1)

        @block.tensor
        def _(tensor: bass.BassTensorEngine):
            for s in range(NSTRIPS):
                b = s % 2
                c, o, sz = STRIPS[s]
                tensor.wait_ge(act_pair, NPAIR * (s + 1))
                tensor.wait_ge(dve_sq, s + 1)
                for g in range(sz):
                    ghi, glo = divmod(g, 64)
                    bb = ghi * (NB2 * 64) + glo
                    aa = ghi * (NA2 * 64) + glo
                    ins = tensor.matmul(
                        ps[:, 2 * T2G * c : 2 * T2G * (c + 1)],
                        plB[b][:, bb : bb + (NB2 - 1) * 64 + 1 : 64],
                        plA[b][:, aa : aa + (NA2 - 1) * 64 + 1 : 64],
                        start=(o == 0 and g == 0),
                        stop=(o + sz == G and g == sz - 1),
                    )
                ins.then_inc(pe_strip, 1)
            tensor.wait_ge(dma_sem, 176)
            for h in range(2):
                ins = tensor.matmul(
                    ph2[:, :],
                    svec[:, h :: 2],
                    consts[:, 1 + h * BINS : 1 + (h + 1) * BINS],
                    start=(h == 0),
                    stop=(h == 1),
                )
            ins.then_inc(pe_sem, 1)
            tensor.wait_ge(dve_sem, 3)
            tensor.matmul(
                pl[0:1, 0:1], ones_sb[0:12, 0:1], ra[0:12, 0:1], start=True, stop=True
            ).then_inc(pe_sem, 1)

        @block.gpsimd
        def _(gpsimd: bass.BassGpSimd):
            gpsimd.wait_ge(dma_sem, 192)
            gpsimd.collective_compute(
                "AllGather",
                OP.bypass,
                replica_groups=[list(range(N_CORES))],
                ins=[cdf_in.ap()],
                outs=[cdf_all.ap()],
            ).then_inc(cc_sem, 1)

    return nc


_VERSION = int(os.environ.get("BASS_HIST_V", "3"))
_NC_CACHE = None


def _get_nc():
    global _NC_CACHE
    if _NC_CACHE is None:
        _NC_CACHE = build_nc_v3() if _VERSION == 3 else build_nc_fourier()
    return _NC_CACHE


def kernel(pred: np.ndarray, target: np.ndarray) -> np.ndarray:
    assert pred.shape == (4, 3, 512, 512) and target.shape == (4, 3, 512, 512)
    chans = np.concatenate(
        [
            np.ascontiguousarray(pred, dtype=np.float32).reshape(12, P, G),
            np.ascontiguousarray(target, dtype=np.float32).reshape(12, P, G),
        ],
        axis=0,
    )  # [24, 128, 2048]

    in_maps = [
        {"x": np.ascontiguousarray(chans[3 * i : 3 * i + 3])} for i in range(N_CORES)
    ]

    nc = _get_nc()
    last_err = None
    for _attempt in range(3):
        try:
            res = run_bass_kernel_spmd(nc, in_maps, core_ids=list(range(N_CORES)))
            out = res.results[0]["out"]
            return np.asarray(out, dtype=np.float32).reshape(())
        except Exception as err:  # transient NRT_EXEC_UNIT_UNRECOVERABLE retries
            last_err = err
    raise last_err


# revision 41
# speedup vs baseline: 2.2190x; 1.0121x over previous
"""ColorHistogramLoss Trainium2 kernel (8 NeuronCores, SPMD).

Sharding: 24 channels total (pred 12 + target 12, channel = (tensor,b,c));
core i owns channels {3i, 3i+1, 3i+2}, each laid out [128, 2048] f32.

v3 (default) - 12-harmonic Fourier factorization, period Pb=70 bins.
The soft histogram hist_j = sum_p exp(-(64 x_p - j - 0.5)^2 / 2) is
approximated from harmonics t = 3*t1 + t2 (t1<4, t2<3) of the periodized
Gaussian: S(t1,t2) = sum_p B_t1(x_p) A_t2(x_p), a bilinear form computed by
the PE in block-diagonal supergroups of M=16 pixel-groups per LDW+MATMUL
(out [8*16, 6*16], useful diagonal blocks accumulated in PSUM; 128
instruction pairs per channel instead of 2048).

Range reduction needs no int casts: the DVE tensor_scalar two-scalar form
fr0 = (x * t*64/70) mod 1 does it in one op; sin/cos come from ScalarE
Sin with args kept in [-pi, pi] via sign absorption into the synthesis
matrix K2 (stored planes are -sin / -cos). A2/B6 planes derive from
A1/B3 by f16 squaring (sin stored halved, compensated in K2); B9 is
direct trig from 3*fr0(B3) mod 1. All planes f16.

Each core AllGathers its raw S block (8x18 f32), then every core runs the
identical tail: one synthesis matmul for all 24 channels, normalize,
cumsum, loss. Simulated accuracy of this exact scheme: rel err ~3e-4.

v2 - previous 42-harmonic version (BASS_HIST_V=2), ~290-360us measured.
"""

import math
import os
from contextlib import ExitStack

import numpy as np

import concourse.bass as bass
import concourse.mybir as mybir
from concourse.bass_utils import run_bass_kernel_spmd

BINS = 64
N_CORES = 8
CH_PER_CORE = 3
P = 128
G = 2048  # 512*512 / 128
F32 = mybir.dt.float32
F16 = mybir.dt.float16
AX = mybir.AxisListType
OP = mybir.AluOpType
AF = mybir.ActivationFunctionType

# ---------------- v3: 12-harmonic scheme ----------------
PB = 70.0
SCALE = 64.0 / PB          # phase per unit x for t=1
NA = 6                     # A planes: (c0,s0,cA1,sA1,cA2,sA2h)
NB = 8                     # B planes: (c0,s0,cB3,sB3,cB6,sB6h,cB9,sB9)
MSG = 16                   # supergroup: groups per LDWEIGHTS+MATMUL
GS3 = 1024                 # plane buffer capacity (groups)
TWO_PI = 2.0 * math.pi
# per-channel strips (offset, size); sizes multiples of 16
CH_STRIPS = [(0, 1024), (1024, 1024)]
STRIPS3 = [(c, o, sz) for c in range(CH_PER_CORE) for (o, sz) in CH_STRIPS]
NSTRIPS3 = len(STRIPS3)
# dma_sem threshold for each strip's x data (ch0a=16, ch0b=32, ch1=48, ch2=64)
def _strip_dma_need(c, o):
    if c == 0:
        return 16 if o == 0 else 32
    return 48 if c == 1 else 64


def _host_k2():
    """Synthesis matrix K2 [48, 64]: row (qb*6+qa), col j.

    hist_j = sum_{t1,t2} Re[w(t) * S(t1,t2)] where S is the bilinear sum of
    TRUE plane values. Stored planes are affine transforms of the true
    values: true = a*stored + b. The b-terms are folded onto the const
    (q0) plane rows. Per plane (a, b):
      q0 const-cos: (1, 0) [value 1]; q1 const-sin: (1, 0) [value 0]
      q2/q3 trig cos/sin: (1, 0)
      q4 derived cos: stored = sin^2(half) -> true = 1 - 2*stored: (-2, 1)
      q5 derived sin: stored = cos*sin = sin(2x)/2 -> (2, 0)
    """
    jj = np.arange(BINS)
    abA = [[(1.0, 0.0)] * 2, [(1.0, 0.0)] * 2, [(-2.0, 1.0), (2.0, 0.0)]]
    abB = [
        [(1.0, 0.0)] * 2,
        [(1.0, 0.0)] * 2,
        [(-2.0, 1.0), (2.0, 0.0)],
        [(1.0, 0.0)] * 2,
    ]
    K = np.zeros((NB, NA, BINS))
    for t1 in range(4):
        for t2 in range(3):
            t = 3 * t1 + t2
            ct = (math.sqrt(2 * math.pi) / PB) * math.exp(
                -0.5 * (2 * math.pi * t / PB) ** 2
            )
            mult = 1.0 if t == 0 else 2.0
            w = mult * ct * np.exp(-1j * 2 * np.pi * t * (0.5 + jj) / PB)
            # coefficients on TRUE products: (cB,cA): Re w; (sB,sA): -Re w;
            # (sB,cA): -Im w; (cB,sA): -Im w
            for (ib, ia), coef in [
                ((0, 0), w.real),
                ((1, 1), -w.real),
                ((1, 0), -w.imag),
                ((0, 1), -w.imag),
            ]:
                qb, qa = 2 * t1 + ib, 2 * t2 + ia
                aB, bB = abB[t1][ib]
                aA, bA = abA[t2][ia]
                K[qb, qa] += coef * aB * aA
                K[qb, 0] += coef * aB * bA
                K[0, qa] += coef * bB * aA
                K[0, 0] += coef * bB * bA
    return np.ascontiguousarray(K.reshape(NB * NA, BINS).astype(np.float32))


def _host_mask():
    """Diagonal mask [128, 288]: 1 where psum col's m' == partition's m."""
    m = np.zeros((P, CH_PER_CORE * NA * MSG), np.float32)
    for p in range(P):
        for col in range(CH_PER_CORE * NA * MSG):
            if col % MSG == p % MSG:
                m[p, col] = 1.0
    return m


def _host_sel():
    """Block selector [128, 8]: SEL[p, qb] = 1 iff p // 16 == qb."""
    s = np.zeros((P, NB), np.float32)
    for p in range(P):
        s[p, p // MSG] = 1.0
    return s


def build_nc_v3():
    nc = bass.Bass(num_devices=N_CORES)

    x_ext = nc.declare_dram_parameter("x", [CH_PER_CORE, P, G], F32, isOutput=False)
    out_ext = nc.declare_dram_parameter("out", [1, 1], F32, isOutput=True)

    cdf_in = nc.dram_tensor("cdf_in", [NB, CH_PER_CORE * NA], F32)
    cdf_all = nc.dram_tensor(
        "cdf_all", [N_CORES * NB, CH_PER_CORE * NA], F32, addr_space="Shared"
    )
    k2_dram = nc.inline_tensor(_host_k2(), name="k2_const")
    mask_dram = nc.inline_tensor(_host_mask(), name="mask_const")
    sel_dram = nc.inline_tensor(_host_sel(), name="sel_const")

    with ExitStack() as stack:
        e = stack.enter_context
        xs = e(nc.sbuf_tensor("xs", [P, CH_PER_CORE * G], F32))
        xh = e(nc.sbuf_tensor("xh", [P, CH_PER_CORE * G], F16))
        # 16-interleaved layout: col = blk*(NP*16) + q*16 + (g%16)
        plA = [e(nc.sbuf_tensor(f"plA{b}", [P, GS3 * NA], F16)) for b in range(2)]
        plB = [e(nc.sbuf_tensor(f"plB{b}", [P, GS3 * NB], F16)) for b in range(2)]
        # args ring: 6 slots/strip (t, t2 for the 3 trig pairs), 2 bufs
        args = e(nc.sbuf_tensor("args", [P, 12 * GS3], F16))
        usc = e(nc.sbuf_tensor("usc", [P, GS3], F16))
        rsc = e(nc.sbuf_tensor("rsc", [P, GS3], F16))
        P3sb = e(nc.sbuf_tensor("P3sb", [P, CH_PER_CORE * NA * MSG], F32))
        R2sb = e(nc.sbuf_tensor("R2sb", [P, CH_PER_CORE * NA], F32))
        masksb = e(nc.sbuf_tensor("masksb", [P, CH_PER_CORE * NA * MSG], F32))
        selsb = e(nc.sbuf_tensor("selsb", [P, NB], F32))
        S3 = e(nc.sbuf_tensor("S3", [NB, CH_PER_CORE * NA], F32))
        SV = e(nc.sbuf_tensor("SV", [NB * NA, N_CORES * CH_PER_CORE], F32))
        k2sb = e(nc.sbuf_tensor("k2sb", [NB * NA, BINS], F32))
        cdfs = e(nc.sbuf_tensor("cdfs", [12, 2 * BINS], F32))
        hsum = e(nc.sbuf_tensor("hsum", [12, 2], F32))
        rinv = e(nc.sbuf_tensor("rinv", [12, 2], F32))
        t1s = e(nc.sbuf_tensor("t1s", [12, BINS], F32))
        ra = e(nc.sbuf_tensor("ra", [12, 1], F32))
        ones_sb = e(nc.sbuf_tensor("ones", [P, 1], F32))
        pospi2 = e(nc.sbuf_tensor("pospi2", [P, 1], F32))
        zerob = e(nc.sbuf_tensor("zerob", [P, 1], F32))
        loss_sb = e(nc.sbuf_tensor("loss", [1, 1], F32))
        ps = e(nc.psum_tensor("ps", [P, CH_PER_CORE * NA * MSG], F32))
        S2 = e(nc.psum_tensor("S2", [NB, CH_PER_CORE * NA], F32))
        ph2 = e(nc.psum_tensor("ph2", [12, 2 * BINS], F32))
        pl = e(nc.psum_tensor("pl", [1, 1], F32))
        dma_sem = e(nc.semaphore("dma_sem"))
        act_xh = e(nc.semaphore("act_xh"))
        dve_pair = e(nc.semaphore("dve_pair"))
        act_pair = e(nc.semaphore("act_pair"))
        dve_sq = e(nc.semaphore("dve_sq"))
        pe_strip = e(nc.semaphore("pe_strip"))
        dve_sem = e(nc.semaphore("dve_sem"))
        pe_sem = e(nc.semaphore("pe_sem"))
        act_sem = e(nc.semaphore("act_sem"))
        cc_sem = e(nc.semaphore("cc_sem"))
        block = e(nc.Block())

        def xv(c, o, sz):
            return xs[:, c * G + o : c * G + o + sz]

        def xhv(c, o, sz):
            return xh[:, c * G + o : c * G + o + sz]

        def pview(pl_, NP, q, sz):
            # plane q over the strip: [p, sz/16, 16] (16-contiguous runs)
            v = pl_.ap().rearrange("p (blk q g) -> p q blk g", q=NP, g=16)
            return v[:, q, 0 : sz // 16, :]

        def slot(s, k, sz):
            base = ((s % 2) * 6 + k) * GS3
            return args[:, base : base + sz]

        @block.sync
        def _(sync: bass.BassEngine):
            sync.dma_start(out=xs[:, 0:1024], in_=x_ext[0, :, 0:1024]).then_inc(
                dma_sem, 16
            )
            sync.dma_start(out=xs[:, 1024:G], in_=x_ext[0, :, 1024:G]).then_inc(
                dma_sem, 16
            )
            sync.dma_start(out=xs[:, G : 2 * G], in_=x_ext[1, :, :]).then_inc(
                dma_sem, 16
            )
            sync.dma_start(out=xs[:, 2 * G : 3 * G], in_=x_ext[2, :, :]).then_inc(
                dma_sem, 16
            )
            sync.dma_start(out=k2sb[:, :], in_=k2_dram[:, :]).then_inc(dma_sem, 16)
            sync.dma_start(out=masksb[:, :], in_=mask_dram[:, :]).then_inc(
                dma_sem, 16
            )
            sync.dma_start(out=selsb[:, :], in_=sel_dram[:, :]).then_inc(dma_sem, 16)
            # raw S block -> collective input
            sync.wait_ge(dve_sem, 2)
            sync.dma_start(out=cdf_in[:, :], in_=S3[0:NB, :]).then_inc(dma_sem, 16)
            # gathered S -> SV [48, 24]: SV[qb*6+qa, core*3+c] = cdf_all[core*8+qb, qa*3+c]
            sync.wait_ge(cc_sem, 1)
            gat = cdf_all.ap().rearrange(
                "(core qb) (qa c) -> (qb qa) core c", qb=NB, c=CH_PER_CORE
            )
            sync.dma_start(out=SV[:, :], in_=gat).then_inc(dma_sem, 16)
            # final scalar
            sync.wait_ge(act_sem, 1)
            sync.dma_start(out=out_ext[:, :], in_=loss_sb[:, :]).then_inc(dma_sem, 16)

        @block.scalar
        def _(scalar: bass.BassScalarEngine):
            def emit_xh(s):
                c, o, sz = STRIPS3[s]
                scalar.wait_ge(dma_sem, _strip_dma_need(c, o))
                scalar.activation(xhv(c, o, sz), xv(c, o, sz), AF.Copy).then_inc(
                    act_xh, 1
                )

            emit_xh(0)
            for s in range(NSTRIPS3):
                c, o, sz = STRIPS3[s]
                b = s % 2
                if s + 1 < NSTRIPS3:
                    emit_xh(s + 1)
                if s >= 2:
                    scalar.wait_ge(pe_strip, s - 1)  # plane buffer reuse
                # trig pair k: cos -> plane q (from t2), sin -> q+1 (from t);
                # for A1/B3 also square the sin plane into q4 (derived cos raw)
                for k, (pl_, NP, q) in enumerate(
                    [(plA[b], NA, 2), (plB[b], NB, 2), (plB[b], NB, 6)]
                ):
                    scalar.wait_ge(dve_pair, 3 * s + k + 1)
                    t = slot(s, 2 * k, sz)
                    t2 = slot(s, 2 * k + 1, sz)
                    tv = t.rearrange("p (blk g) -> p blk g", g=16)
                    t2v = t2.rearrange("p (blk g) -> p blk g", g=16)
                    scalar.activation(
                        pview(pl_, NP, q, sz), t2v, AF.Sin,
                        bias=pospi2[:, 0:1], scale=TWO_PI,
                    )
                    ins = scalar.activation(
                        pview(pl_, NP, q + 1, sz), tv, AF.Sin,
                        bias=zerob[:, 0:1], scale=TWO_PI,
                    )
                    if k < 2:
                        ins = scalar.square(
                            pview(pl_, NP, 4, sz), pview(pl_, NP, 3, sz)
                        )
                    ins.then_inc(act_pair, 1)
            # final: loss = pl / 768
            scalar.wait_ge(pe_sem, 3)
            scalar.mul(loss_sb[:, :], pl[:, :], 1.0 / 768.0).then_inc(act_sem, 1)

        @block.vector
        def _(vector: bass.BassVectorEngine):
            vector.memset(ones_sb[:, :], 1.0)
            vector.memset(pospi2[:, :], math.pi / 2.0)
            vector.memset(zerob[:, :], 0.0)
            for b in range(2):
                vector.memset(pview(plA[b], NA, 0, GS3), 1.0)
                vector.memset(pview(plA[b], NA, 1, GS3), 0.0)
                vector.memset(pview(plB[b], NB, 0, GS3), 1.0)
                vector.memset(pview(plB[b], NB, 1, GS3), 0.0)
            for s in range(NSTRIPS3):
                c, o, sz = STRIPS3[s]
                b = s % 2
                vector.wait_ge(act_xh, s + 1)
                if s >= 2:
                    vector.wait_ge(act_pair, 3 * (s - 1))  # args ring reuse
                    vector.wait_ge(pe_strip, s - 1)  # plane buffer reuse (sq)
                xin = xhv(c, o, sz)
                u = usc[:, 0:sz]
                r = rsc[:, 0:sz]
                for k, ts in enumerate((SCALE, 3 * SCALE, 9 * SCALE)):
                    t = slot(s, 2 * k, sz)
                    t2 = slot(s, 2 * k + 1, sz)
                    vector.tensor_scalar(u, xin, ts, None, OP.mult)
                    # r = round(u) via f16 magic add (output rounding), -1536
                    vector.tensor_scalar(r, u, 1536.0, None, OP.add)
                    vector.tensor_scalar(r, r, -1536.0, None, OP.add)
                    vector.tensor_sub(t, u, r)
                    # t2 = t - (t >= 0.25): phase for the cos plane
                    vector.tensor_scalar(r, t, 0.25, None, OP.is_ge)
                    vector.tensor_sub(t2, t, r).then_inc(dve_pair, 1)
                # derived sin planes: c*s raw in q5 (s^2 done on ScalarE);
                # the 1-2s^2 / doubling transforms are folded into K2
                for j, (pl_, NP) in enumerate([(plA[b], NA), (plB[b], NB)]):
                    vector.wait_ge(act_pair, 3 * s + j + 1)
                    cst = pview(pl_, NP, 2, sz)
                    sst = pview(pl_, NP, 3, sz)
                    vector.tensor_mul(pview(pl_, NP, 5, sz), cst, sst).then_inc(
                        dve_sq, 1
                    )
            # ---- tail: masked diagonal extraction from PSUM ----
            vector.wait_ge(pe_strip, NSTRIPS3)
            vector.wait_ge(dma_sem, 112)  # mask + sel consts loaded
            vector.tensor_mul(P3sb[:, :], ps[:, :], masksb[:, :])
            p3r = P3sb.ap().rearrange(
                "p (c qa m) -> p qa c m", c=CH_PER_CORE, m=MSG
            )
            r2v = R2sb.ap().rearrange("p (qa c) -> p qa c", c=CH_PER_CORE)
            vector.tensor_reduce(r2v, p3r, AX.X, OP.add).then_inc(dve_sem, 1)
            # S2 (psum) -> S3 (sbuf) once the selector matmul lands
            vector.wait_ge(pe_sem, 1)
            vector.tensor_copy(S3[:, :], S2[:, :]).then_inc(dve_sem, 1)
            # ---- post-gather: normalize + cumsum + loss (identical on all cores)
            # ph2 is [12, 2*64]: pred hists in cols 0:64, target in 64:128
            vector.wait_ge(pe_sem, 2)
            vector.tensor_reduce(
                hsum[:, :],
                ph2.ap().rearrange("p (h j) -> p h j", h=2),
                AX.X,
                OP.add,
            )
            vector.scalar_tensor_tensor(
                hsum[:, :], hsum[:, :], 1e-8, hsum[:, :], OP.add, OP.bypass
            )
            vector.reciprocal(rinv[:, :], hsum[:, :])
            for h in range(2):
                vector.tensor_tensor_scan(
                    cdfs[:, h * BINS : (h + 1) * BINS],
                    ph2[:, h * BINS : (h + 1) * BINS],
                    P3sb[0:12, 0:BINS],
                    0.0,
                    OP.add,
                    OP.bypass,
                )
            vector.scalar_tensor_tensor(
                t1s[:, :], cdfs[:, 0:BINS], rinv[:, 0:1], cdfs[:, 0:BINS],
                OP.mult, OP.bypass,
            )
            vector.scalar_tensor_tensor(
                t1s[:, :], cdfs[:, BINS : 2 * BINS], rinv[:, 1:2], t1s[:, :],
                OP.mult, OP.subtract,
            )
            vector.tensor_reduce(
                ra[:, :], t1s[:, :], AX.X, OP.add, apply_absolute_value=True
            ).then_inc(dve_sem, 1)

        @block.tensor
        def _(tensor: bass.BassTensorEngine):
            for s in range(NSTRIPS3):
                c, o, sz = STRIPS3[s]
                b = s % 2
                tensor.wait_ge(act_pair, 3 * (s + 1))
                tensor.wait_ge(dve_sq, 2 * (s + 1))
                nsg = sz // MSG
                for j in range(nsg):
                    ins = tensor.matmul(
                        ps[:, c * NA * MSG : (c + 1) * NA * MSG],
                        plB[b][:, j * MSG * NB : (j + 1) * MSG * NB],
                        plA[b][:, j * MSG * NA : (j + 1) * MSG * NA],
                        start=(o == 0 and j == 0),
                        stop=(o + sz == G and j == nsg - 1),
                    )
                ins.then_inc(pe_strip, 1)
            # selector matmul collapses the 16 diagonal blocks: S2 = SEL^T @ R2
            tensor.wait_ge(dve_sem, 1)
            tensor.matmul(
                S2[:, :], selsb[:, :], R2sb[:, :], start=True, stop=True
            ).then_inc(pe_sem, 1)
            # synthesis: pred channels -> ph2 cols 0:64, target -> 64:128
            tensor.wait_ge(dma_sem, 144)
            tensor.matmul(
                ph2[:, 0:BINS], SV[:, 0:12], k2sb[:, :], start=True, stop=True
            )
            tensor.matmul(
                ph2[:, BINS : 2 * BINS], SV[:, 12:24], k2sb[:, :],
                start=True, stop=True,
            ).then_inc(pe_sem, 1)
            tensor.wait_ge(dve_sem, 3)
            tensor.matmul(
                pl[0:1, 0:1], ones_sb[0:12, 0:1], ra[0:12, 0:1],
                start=True, stop=True,
            ).then_inc(pe_sem, 1)

        @block.gpsimd
        def _(gpsimd: bass.BassGpSimd):
            gpsimd.wait_ge(dma_sem, 128)
            gpsimd.collective_compute(
                "AllGather",
                OP.bypass,
                replica_groups=[list(range(N_CORES))],
                ins=[cdf_in.ap()],
                outs=[cdf_all.ap()],
            ).then_inc(cc_sem, 1)

    return nc


# ---------------- v2: 42-harmonic fallback (previous session) ----------------
T1G = 6
T2G = 7
NA2 = 2 * T2G
NB2 = 2 * T1G
BPAD = 16
PACK = BINS + 1


def _host_k_matrix():
    Pb = 64 * 1.25
    K = np.zeros((2 * T1G, BPAD, BINS), np.float64)
    j = np.arange(BINS)
    for t1 in range(T1G):
        for t2 in range(T2G):
            t = t1 * T2G + t2
            ct = (math.sqrt(2 * math.pi) / Pb) * math.exp(
                -0.5 * (2 * math.pi * t / Pb) ** 2
            )
            mult = 1.0 if t == 0 else 2.0
            w = mult * ct * np.exp(-1j * 2 * np.pi * t * (0.5 + j) / Pb)
            K[2 * t1, 2 * t2] = w.real
            K[2 * t1 + 1, 2 * t2 + 1] = -w.real
            K[2 * t1, 2 * t2 + 1] = -w.imag
            K[2 * t1 + 1, 2 * t2, :] = -w.imag
    K[9, :, :] *= 2.0
    K[:, 9, :] *= 2.0
    K[:, 13, :] *= 2.0
    consts = np.zeros((128, 129), np.float32)
    for p in range(128):
        for h in range(2):
            a = h * 8 + p // BPAD
            b = p % BPAD
            if a < 2 * T1G:
                consts[p, 1 + h * 64 : 1 + (h + 1) * 64] = K[a, b]
    return consts


GS = 1024
STRIPS = [(0, 0, 256), (0, 256, 256), (0, 512, 512), (0, 1024, 1024)] + [
    (1, 0, 1024), (1, 1024, 1024),
    (2, 0, 1024), (2, 1024, 512), (2, 1536, 512),
]
NSTRIPS = len(STRIPS)
NPAIR = 6


def build_nc_fourier():
    nc = bass.Bass(num_devices=N_CORES)

    x_ext = nc.declare_dram_parameter("x", [CH_PER_CORE, P, G], F32, isOutput=False)
    out_ext = nc.declare_dram_parameter("out", [1, 1], F32, isOutput=True)

    cdf_in = nc.dram_tensor("cdf_in", [1, CH_PER_CORE * PACK], F32)
    cdf_all = nc.dram_tensor(
        "cdf_all", [N_CORES, CH_PER_CORE * PACK], F32, addr_space="Shared"
    )
    sdram = nc.dram_tensor("sdram", [2 * T1G, CH_PER_CORE, BPAD], F32)
    consts_dram = nc.inline_tensor(_host_k_matrix(), name="consts_k")

    pairs = [
        (1, "A", 2),
        (T2G, "B", 2),
        (3, "A", 6),
        (3 * T2G, "B", 6),
        (5, "A", 10),
        (5 * T2G, "B", 10),
    ]
    squares = [
        ("A", 2, 4, False, 0),
        ("B", 2, 4, False, 1),
        ("A", 6, 12, True, 2),
        ("A", 4, 8, True, None),
        ("B", 4, 8, True, None),
    ]

    with ExitStack() as stack:
        e = stack.enter_context
        xs = e(nc.sbuf_tensor("xs", [P, CH_PER_CORE * G], F32))
        consts = e(nc.sbuf_tensor("consts", [P, 129], F32))
        plA = [e(nc.sbuf_tensor(f"plA{b}", [P, NA2 * GS], F16)) for b in range(2)]
        plB = [e(nc.sbuf_tensor(f"plB{b}", [P, NB2 * GS], F16)) for b in range(2)]
        args = e(nc.sbuf_tensor("args", [P, 8 * GS], F32))
        irnd = e(nc.sbuf_tensor("irnd", [P, 2 * GS], mybir.dt.int32))
        i1r = e(nc.sbuf_tensor("i1r", [P, 4 * GS], mybir.dt.int32))
        sqsc = e(nc.sbuf_tensor("sqsc", [P, 2 * GS], F16))
        ones_sb = e(nc.sbuf_tensor("ones", [P, 1], F32))
        S_sb = e(nc.sbuf_tensor("S_sb", [2 * T1G, CH_PER_CORE * BPAD], F32))
        svec = e(nc.sbuf_tensor("svec", [P, 2 * CH_PER_CORE], F32))
        hsum = e(nc.sbuf_tensor("hsum", [CH_PER_CORE, 1], F32))
        rinv = e(nc.sbuf_tensor("rinv", [CH_PER_CORE, 1], F32))
        packed = e(nc.sbuf_tensor("packed", [CH_PER_CORE, PACK], F32))
        gp = e(nc.sbuf_tensor("gp", [12, PACK], F32))
        gt = e(nc.sbuf_tensor("gt", [12, PACK], F32))
        t1s = e(nc.sbuf_tensor("t1s", [12, BINS], F32))
        t2s = e(nc.sbuf_tensor("t2s", [12, BINS], F32))
        ra = e(nc.sbuf_tensor("ra", [12, 1], F32))
        loss_sb = e(nc.sbuf_tensor("loss", [1, 1], F32))
        ps = e(nc.psum_tensor("ps", [2 * T1G, CH_PER_CORE * 2 * T2G], F32))
        ph2 = e(nc.psum_tensor("ph2", [CH_PER_CORE, BINS], F32))
        pl = e(nc.psum_tensor("pl", [1, 1], F32))
        dma_sem = e(nc.semaphore("dma_sem"))
        dve_pair = e(nc.semaphore("dve_pair"))
        act_pair = e(nc.semaphore("act_pair"))
        act_i1 = e(nc.semaphore("act_i1"))
        pe_strip = e(nc.semaphore("pe_strip"))
        dve_sq = e(nc.semaphore("dve_sq"))
        pe_sem = e(nc.semaphore("pe_sem"))
        dve_sem = e(nc.semaphore("dve_sem"))
        act_sem = e(nc.semaphore("act_sem"))
        cc_sem = e(nc.semaphore("cc_sem"))
        block = e(nc.Block())

        def strip_x(s):
            c, o, sz = STRIPS[s]
            return xs[:, c * G + o : c * G + o + sz]

        def pview(pl_, NP, q, sz):
            v = pl_.ap().rearrange("p (ghi q glo) -> p q ghi glo", q=NP, glo=64)
            return v[:, q, 0 : sz // 64, :]

        def blk(ap2d):
            return ap2d.rearrange("p (a b) -> p a b", b=64)

        @block.sync
        def _(sync: bass.BassEngine):
            sync.dma_start(out=consts[:, :], in_=consts_dram[:, :]).then_inc(
                dma_sem, 16
            )
            for c in range(CH_PER_CORE):
                sync.dma_start(
                    out=xs[:, c * G : (c + 1) * G], in_=x_ext[c, :, :]
                ).then_inc(dma_sem, 16)
            sync.wait_ge(dve_sem, 1)
            sync.dma_start(out=sdram[:, :, :], in_=S_sb[:, :]).then_inc(dma_sem, 16)
            for c in range(CH_PER_CORE):
                for h in range(2):
                    lo = h * 8
                    hi = min(2 * T1G, lo + 8)
                    sync.dma_start(
                        out=svec[0 : (hi - lo) * BPAD, 2 * c + h : 2 * c + h + 1],
                        in_=sdram[lo:hi, c, :],
                    ).then_inc(dma_sem, 16)
            sync.wait_ge(dve_sem, 2)
            sync.dma_start(out=cdf_in[:, :], in_=packed[:, :]).then_inc(dma_sem, 16)
            sync.wait_ge(cc_sem, 1)
            sync.dma_start(out=gp[:, :], in_=cdf_all[0:4, :]).then_inc(dma_sem, 16)
            sync.dma_start(out=gt[:, :], in_=cdf_all[4:8, :]).then_inc(dma_sem, 16)
            sync.wait_ge(act_sem, 1)
            sync.dma_start(out=out_ext[:, :], in_=loss_sb[:, :]).then_inc(dma_sem, 16)

        @block.vector
        def _(vector: bass.BassVectorEngine):
            vector.memset(ones_sb[:, :], 1.0)
            vector.memset(svec[:, :], 0.0)
            vector.memset(S_sb[:, :], 0.0)
            for b in range(2):
                vector.memset(pview(plA[b], NA2, 0, GS), 1.0)
                vector.memset(pview(plA[b], NA2, 1, GS), 0.0)
                vector.memset(pview(plB[b], NB2, 0, GS), 1.0)
                vector.memset(pview(plB[b], NB2, 1, GS), 0.0)
            vector.wait_ge(dma_sem, 64)
            for s in range(NSTRIPS):
                xin = strip_x(s)
                b = s % 2
                sz = STRIPS[s][2]

                def do_square(job, inc=False):
                    side, sq_, dq, halve, wk = job
                    pl_ = plA[b] if side == "A" else plB[b]
                    NP = NA2 if side == "A" else NB2
                    if wk is not None:
                        vector.wait_ge(act_pair, s * NPAIR + wk + 1)
                    sc = pview(pl_, NP, sq_, sz)
                    sn = pview(pl_, NP, sq_ + 1, sz)
                    dc = pview(pl_, NP, dq, sz)
                    ds = pview(pl_, NP, dq + 1, sz)
                    pair = pl_.ap().rearrange(
                        "p (ghi qq glo) -> p qq ghi glo", qq=NP // 2, glo=128
                    )[:, sq_ // 2, 0 : sz // 64, :]
                    sq2 = sqsc.ap().rearrange("p (a b) -> p a b", b=128)[
                        :, 0 : sz // 64, :
                    ]
                    vector.tensor_mul(sq2, pair, pair)
                    vector.tensor_sub(dc, sq2[:, :, 0:64], sq2[:, :, 64:128])
                    ins = vector.tensor_mul(ds, sc, sn)
                    if not halve:
                        ins = vector.tensor_scalar(ds, ds, 2.0, None, OP.mult)
                    if inc:
                        ins.then_inc(dve_sq, 1)

                for k, (t, _, _) in enumerate(pairs):
                    kg = s * NPAIR + k
                    if kg >= 4:
                        vector.wait_ge(act_pair, kg - 3)
                    vector.wait_ge(act_i1, kg + 1)
                    slot = (kg % 4) * 2
                    fr = args[:, slot * GS : slot * GS + sz]
                    frc = args[:, (slot + 1) * GS : (slot + 1) * GS + sz]
                    ir = irnd[:, (kg % 2) * GS : (kg % 2) * GS + sz]
                    i1 = i1r[:, (kg % 4) * GS : (kg % 4) * GS + sz]
                    vector.scalar_tensor_tensor(
                        fr, xin, t * 0.8, i1, OP.mult, OP.subtract
                    )
                    vector.tensor_scalar(ir, fr, 0.25, None, OP.add)
                    vector.scalar_tensor_tensor(
                        frc, fr, 0.25, ir, OP.add, OP.subtract
                    ).then_inc(dve_pair, 1)
                    if k == 2:
                        if s >= 2:
                            vector.wait_ge(pe_strip, s - 1)
                        do_square(squares[0])
                        do_square(squares[1])
                        do_square(squares[3])
                        do_square(squares[4])
                    elif k == 4:
                        do_square(squares[2], inc=True)
            vector.wait_ge(pe_strip, NSTRIPS)
            for c in range(CH_PER_CORE):
                ins = vector.tensor_copy(
                    S_sb[:, c * BPAD : c * BPAD + 2 * T2G],
                    ps[:, c * 2 * T2G : (c + 1) * 2 * T2G],
                )
            ins.then_inc(dve_sem, 1)
            vector.wait_ge(pe_sem, 1)
            vector.tensor_reduce(hsum[:, :], ph2[:, :], AX.X, OP.add)
            vector.scalar_tensor_tensor(
                hsum[:, :], hsum[:, :], 1e-8, hsum[:, :], OP.add, OP.bypass
            )
            vector.reciprocal(rinv[:, :], hsum[:, :])
            vector.tensor_tensor_scan(
                packed[:, 0:BINS],
                ph2[:, :],
                t1s[0:CH_PER_CORE, :],
                0.0,
                OP.add,
                OP.bypass,
            )
            vector.tensor_copy(packed[:, BINS : BINS + 1], rinv[:, :]).then_inc(
                dve_sem, 1
            )
            vector.wait_ge(dma_sem, 224)
            vector.scalar_tensor_tensor(
                t1s[:, :],
                gp[:, 0:BINS],
                gp[:, BINS : BINS + 1],
                gp[:, 0:BINS],
                OP.mult,
                OP.bypass,
            )
            vector.scalar_tensor_tensor(
                t2s[:, :],
                gt[:, 0:BINS],
                gt[:, BINS : BINS + 1],
                t1s[:, :],
                OP.mult,
                OP.subtract,
            )
            vector.tensor_reduce(
                ra[:, :], t2s[:, :], AX.X, OP.add, apply_absolute_value=True
            ).then_inc(dve_sem, 1)

        @block.scalar
        def _(scalar: bass.BassScalarEngine):
            scalar.wait_ge(dma_sem, 64)
            NKG = NSTRIPS * NPAIR
            all_pairs = [(s, k) + pairs[k] for s in range(NSTRIPS) for k in range(NPAIR)]

            def emit_i1(kg):
                s, k, t, _, _ = all_pairs[kg]
                sz = STRIPS[s][2]
                i1 = i1r[:, (kg % 4) * GS : (kg % 4) * GS + sz]
                return scalar.activation(
                    i1, strip_x(s), AF.Copy, bias=0.0, scale=t * 0.8
                ).then_inc(act_i1, 1)

            emit_i1(0)
            emit_i1(1)
            for kg in range(NKG):
                s, k, t, which, q = all_pairs[kg]
                b = s % 2
                if kg + 2 < NKG:
                    if kg >= 2:
                        scalar.wait_ge(dve_pair, kg - 1)
                    emit_i1(kg + 2)
                if k == 0 and s >= 2:
                    scalar.wait_ge(pe_strip, s - 1)
                scalar.wait_ge(dve_pair, kg + 1)
                sz = STRIPS[s][2]
                slot = (kg % 4) * 2
                fr = args[:, slot * GS : slot * GS + sz]
                frc = args[:, (slot + 1) * GS : (slot + 1) * GS + sz]
                pl_ = plA[b] if which == "A" else plB[b]
                NP = NA2 if which == "A" else NB2
                scalar.activation(
                    pview(pl_, NP, q, sz),
                    blk(frc),
                    AF.Sin,
                    bias=consts[:, 0:1],
                    scale=TWO_PI,
                )
                scalar.activation(
                    pview(pl_, NP, q + 1, sz),
                    blk(fr),
                    AF.Sin,
                    bias=consts[:, 0:1],
                    scale=TWO_PI,
                ).then_inc(act_pair, 1)
            scalar.wait_ge(pe_sem, 2)
            scalar.mul(loss_sb[:, :], pl[:, :], 1.0 / 768.0).then_inc(act_sem, 1)

        @block.tensor
        def _(tensor: bass.BassTensorEngine):
            for s in range(NSTRIPS):
                b = s % 2
                c, o, sz = STRIPS[s]
                tensor.wait_ge(act_pair, NPAIR * (s + 1))
                tensor.wait_ge(dve_sq, s + 1)
                for g in range(sz):
                    ghi, glo = divmod(g, 64)
                    bb = ghi * (NB2 * 64) + glo
                    aa = ghi * (NA2 * 64) + glo
                    ins = tensor.matmul(
                        ps[:, 2 * T2G * c : 2 * T2G * (c + 1)],
                        plB[b][:, bb : bb + (NB2 - 1) * 64 + 1 : 64],
                        plA[b][:, aa : aa + (NA2 - 1) * 64 + 1 : 64],
                        start=(o == 0 and g == 0),
                        stop=(o + sz == G and g == sz - 1),
                    )
                ins.then_inc(pe_strip, 1)
            tensor.wait_ge(dma_sem, 176)
            for h in range(2):
                ins = tensor.matmul(
                    ph2[:, :],
                    svec[:, h :: 2],
                    consts[:, 1 + h * BINS : 1 + (h + 1) * BINS],
                    start=(h == 0),
                    stop=(h == 1),
                )
            ins.then_inc(pe_sem, 1)
            tensor.wait_ge(dve_sem, 3)
            tensor.matmul(
                pl[0:1, 0:1], ones_sb[0:12, 0:1], ra[0:12, 0:1], start=True, stop=True
            ).then_inc(pe_sem, 1)

        @block.gpsimd
        def _(gpsimd: bass.BassGpSimd):
            gpsimd.wait_ge(dma_sem, 192)
            gpsimd.collective_compute(
                "AllGather",
                OP.bypass,
                replica_groups=[list(range(N_CORES))],
                ins=[cdf_in.ap()],
                outs=[cdf_all.ap()],
            ).then_inc(cc_sem, 1)

    return nc


_VERSION = int(os.environ.get("BASS_HIST_V", "3"))
_NC_CACHE = None


def _get_nc():
    global _NC_CACHE
    if _NC_CACHE is None:
        _NC_CACHE = build_nc_v3() if _VERSION == 3 else build_nc_fourier()
    return _NC_CACHE


def kernel(pred: np.ndarray, target: np.ndarray) -> np.ndarray:
    assert pred.shape == (4, 3, 512, 512) and target.shape == (4, 3, 512, 512)
    chans = np.concatenate(
        [
            np.ascontiguousarray(pred, dtype=np.float32).reshape(12, P, G),
            np.ascontiguousarray(target, dtype=np.float32).reshape(12, P, G),
        ],
        axis=0,
    )  # [24, 128, 2048]

    in_maps = [
        {"x": np.ascontiguousarray(chans[3 * i : 3 * i + 3])} for i in range(N_CORES)
    ]

    nc = _get_nc()
    last_err = None
    for _attempt in range(3):
        try:
            res = run_bass_kernel_spmd(nc, in_maps, core_ids=list(range(N_CORES)))
            out = res.results[0]["out"]
            return np.asarray(out, dtype=np.float32).reshape(())
        except Exception as err:  # transient NRT_EXEC_UNIT_UNRECOVERABLE retries
            last_err = err
    raise last_err
